# revision 1
# baseline (speedup 1.0000x reference)
"""DRCLoss kernel v3 for 8 Trainium2 NeuronCores (Bass/Tile, SPMD).

Math: loss = mean_i[ relu(l1_i + l2_i + d12_i - neg_i + 0.1) + max(l1_i, l2_i) ]
  where neg_i = min over non-self columns of cdist(ts, [ts; im1; im2])[i, :].

v3 strategy (column-sharded: each core owns 1536 of the 12288 columns j,
all 4096 rows i):
  - Device computes s[j, i] = 2*r_j.x_i - rsq[j] with output partitions = j.
    Only the masked column-min survives on device; l1/l2/d12/xsq are exact
    host math, so the output is just the running row-max of s.
  - Stationary (R-chunk) is reused across all 8 i-blocks of a j-tile; PSUM
    is used as two 4-bank [128, 2048] quads, so one fused op drains 4 tiles.
  - Per quad, one of two fold paths:
      D: DVE scalar_tensor_tensor  acc = max(acc, psum + (-rsq_j))  (fp32)
      A: ScalarE activation copy psum+bias -> fp16, DVE tensor_max fold
    balancing the drain work across ScalarE and DVE.
  - Self-column exclusion: each j-tile has one 128-long self diagonal. A
    per-core rotation of the i axis (shift by 1536c) makes the diagonal
    position uniform across cores, so a single eye x shifted-identity bf16
    matmul per j-tile adds -30000 in PSUM (SPMD-safe, no vector-engine cost).
  - Host finishes: negsq = xsq - max over cores/partitions, sqrt/relu/mean
    in float64.
"""

import sys

if "/opt/trn_rl_repo" not in sys.path:
    sys.path.insert(0, "/opt/trn_rl_repo")

from contextlib import ExitStack

import ml_dtypes
import numpy as np

import concourse.bass as bass
import concourse.tile as tile
from concourse import mybir
from concourse.bass_utils import run_bass_kernel_spmd

BF16 = ml_dtypes.bfloat16
F8 = ml_dtypes.float8_e4m3

B = 4096          # rows
D = 512           # feature dim
M = 8             # cores
JC = 3 * B // M   # columns per core (1536)
NJT = JC // 128   # j-tiles per core (12)
IH = B // 2       # rows per half (2048)

# quads folded via the D path (DVE stt direct from PSUM); rest via A path
# (ScalarE bias-copy + DVE fp16 fold). 6 of 24 per the engine balance.
D_QUADS = {(1, 3), (1, 7), (1, 11), (0, 3), (0, 7), (0, 11)}

LAST_RESULTS = None

_NC_CACHE = None


def _install_ntff_hook():
    """Provide antenv.axon_hooks (missing in this image) so trace=True can
    capture NTFF profiles through libaxon_pjrt.so."""
    try:
        import antenv.axon_hooks  # noqa: F401

        return
    except ImportError:
        pass
    try:
        import types

        import antenv
        from trn_agent_boot.trn_boot import _ntff_profile_via_ctypes

        mod = types.ModuleType("antenv.axon_hooks")
        mod._hook = None

        def set_axon_ntff_profile_hook(h):
            mod._hook = h

        def get_axon_ntff_profile_hook():
            return mod._hook

        mod.set_axon_ntff_profile_hook = set_axon_ntff_profile_hook
        mod.get_axon_ntff_profile_hook = get_axon_ntff_profile_hook
        sys.modules["antenv.axon_hooks"] = mod
        antenv.axon_hooks = mod
        hook = _ntff_profile_via_ctypes("/opt/axon/libaxon_pjrt.so")
        if hook is not None:
            mod._hook = hook
    except Exception:
        pass


def _split_multi_waits(nc):
    """This walrus build allows only ONE embedded sync wait per instruction.
    Hoist extra waits onto standalone EventSemaphore instructions inserted
    just before the owner (same engine, so program order is preserved)."""
    import bass_rust

    ctr = 0
    for blk in nc.m.functions[0].blocks:
        il = blk.instructions
        new = []
        for inst in il:
            si = getattr(inst, "sync_info", None)
            waits = list(si.on_wait) if si is not None else []
            if len(waits) > 1:
                for w in waits[:-1]:
                    ev = bass_rust.InstEventSemaphore(name=f"wsplit_{ctr}")
                    ctr += 1
                    ev.engine = inst.engine
                    ev.sync_info = bass_rust.SyncInfo(on_wait=[w], on_update=[])
                    new.append(ev)
                inst.sync_info = bass_rust.SyncInfo(
                    on_wait=[waits[-1]], on_update=list(si.on_update)
                )
            new.append(inst)
        il[:] = new


def _build_nc():
    nc = bass.Bass()
    f32 = mybir.dt.float32
    f16 = mybir.dt.float16
    bf16 = mybir.dt.bfloat16
    f8 = mybir.dt.float8e4
    DR = mybir.MatmulPerfMode.DoubleRow
    add = mybir.AluOpType.add
    mx = mybir.AluOpType.max

    # stationary R-chunks [pk, jt, kh, dr, j]
    st_d = nc.dram_tensor("st8", [128, NJT, 2, 2, 128], f8, kind="ExternalInput")
    # moving X^T, i-rotated per core, ihalf-major [pk, ih, kh, dr, i]
    xt_d = nc.dram_tensor("xt8", [128, 2, 2, 2, IH], f8, kind="ExternalInput")
    rsqT_d = nc.dram_tensor("rsqT", [128, NJT], f32, kind="ExternalInput")
    mask_d = nc.dram_tensor("mask", [128, 4, 2, 512], f8, kind="ExternalInput")
    eye_d = nc.dram_tensor("eye", [128, 2, 128], f8, kind="ExternalInput")
    oD_d = nc.dram_tensor("oD", [128, B], f32, kind="ExternalOutput")
    oA_d = nc.dram_tensor("oA", [128, B], f16, kind="ExternalOutput")

    with ExitStack() as ctx:
        tc = ctx.enter_context(tile.TileContext(nc))
        const = ctx.enter_context(tc.tile_pool(name="const", bufs=1))
        hpp = ctx.enter_context(tc.tile_pool(name="hp", bufs=3))
        psump = ctx.enter_context(tc.tile_pool(name="psum", bufs=2, space="PSUM"))

        def dummy_mm(lhs_ap, rhs_ap):
            pw = psump.tile([128, 2048], f32, tag="q", name="pdum")
            nc.tensor.matmul(pw[: lhs_ap.shape[-1], : rhs_ap.shape[-1]],
                             lhs_ap, rhs_ap, start=True, stop=True)

        # ihalf=1 moving slice arrives first (processed first, mask-free)
        xt1 = const.tile([128, 2, 2, IH], f8, tag="xt1")
        nc.sync.dma_start(out=xt1, in_=xt_d[:, 1])
        dummy_mm(xt1[:, 0, 0, 0:4], xt1[:, 0, 0, 0:8])
        # short warmup burst to shake the PE HAM throttle while DMAs land
        for _ in range(8):
            pw = psump.tile([128, 2048], f32, tag="q", name="pwarm")
            nc.tensor.matmul(pw[:, 0:512], xt1[:, 0, 0, 0:128], xt1[:, 0, 0, 0:512],
                             start=True, stop=True)

        st = const.tile([128, NJT, 2, 2, 128], f8, tag="st")
        nc.sync.dma_start(out=st, in_=st_d[:, :])
        dummy_mm(st[:, 0, 0, 0, 0:4], st[:, 0, 0, 0, 0:8])

        rsqT = const.tile([128, NJT], f32, tag="rsqT")
        nc.sync.dma_start(out=rsqT, in_=rsqT_d[:, :])
        vabs = const.tile([128, 1], f32, tag="vabs")
        nc.vector.tensor_copy(vabs, rsqT[:, 0:1])
        sabs = const.tile([128, 1], f32, tag="sabs")
        nc.scalar.copy(sabs, rsqT[:, 0:1])

        xt0 = const.tile([128, 2, 2, IH], f8, tag="xt0")
        nc.sync.dma_start(out=xt0, in_=xt_d[:, 0])
        dummy_mm(xt0[:, 0, 0, 0:4], xt0[:, 0, 0, 0:8])

        maskr = const.tile([128, 4, 2, 512], f8, tag="maskr")
        nc.sync.dma_start(out=maskr, in_=mask_d[:, :])
        dummy_mm(maskr[:, 0, 0, 0:4], maskr[:, 0, 0, 0:8])
        eye = const.tile([128, 2, 128], f8, tag="eye")
        nc.sync.dma_start(out=eye, in_=eye_d[:, :])
        dummy_mm(eye[:, 0, 0:4], eye[:, 0, 0:8])

        accD = const.tile([128, B], f32, tag="accD")
        nc.gpsimd.memset(accD, -60000.0)
        accA = const.tile([128, B], f16, tag="accA")
        nc.vector.memset(accA, -60000.0)

        for ihalf in (1, 0):
            xt = xt1 if ihalf == 1 else xt0
            has_mask = ihalf == 0
            dsl = accD[:, ihalf * IH : (ihalf + 1) * IH]
            asl = accA[:, ihalf * IH : (ihalf + 1) * IH]
            for jt in range(NJT):
                q = psump.tile([128, 2048], f32, tag="q", name="q")
                mib = jt // 4 if has_mask else -1
                for kh in range(2):
                    for ib in range(4):
                        nc.tensor.matmul(
                            q[:, ib * 512 : (ib + 1) * 512],
                            st[:, jt, kh],
                            xt[:, kh, :, ib * 512 : (ib + 1) * 512],
                            start=(kh == 0),
                            stop=(kh == 1 and ib != mib),
                            perf_mode=DR,
                        )
                if has_mask:
                    nc.tensor.matmul(
                        q[:, mib * 512 : (mib + 1) * 512],
                        eye,
                        maskr[:, jt % 4],
                        start=False,
                        stop=True,
                        perf_mode=DR,
                    )
                if (ihalf, jt) in D_QUADS:
                    nc.vector.scalar_tensor_tensor(
                        out=dsl, in0=q, scalar=rsqT[:, jt : jt + 1], in1=dsl,
                        op0=add, op1=mx,
                    )
                else:
                    hp = hpp.tile([128, 2048], f16, tag="hp")
                    nc.scalar.add(hp, q, rsqT[:, jt : jt + 1])
                    nc.vector.tensor_max(asl, asl, hp)
            # stream this half's results out while the other half computes
            nc.gpsimd.dma_start(out=oD_d[:, ihalf * IH : (ihalf + 1) * IH], in_=dsl)
            nc.sync.dma_start(out=oA_d[:, ihalf * IH : (ihalf + 1) * IH], in_=asl)

    _split_multi_waits(nc)
    return nc


def _host_inputs(feature_ts, feature_image1, feature_image2):
    ts = np.ascontiguousarray(feature_ts, dtype=np.float32)
    im1 = np.ascontiguousarray(feature_image1, dtype=np.float32)
    im2 = np.ascontiguousarray(feature_image2, dtype=np.float32)

    R = np.concatenate([ts, im1, im2], 0)                      # [3B, D]
    rsq = (R.astype(np.float64) ** 2).sum(1)                   # [3B]
    x8 = ts.astype(F8)                                         # [B, D]
    r2_8 = (2.0 * R.astype(F8).astype(np.float32)).astype(F8)  # exact doubling

    maskr = np.zeros((128, 4, 2, 512), dtype=F8)
    p = np.arange(64)
    for toff in range(4):
        for dr in range(2):
            maskr[p, toff, dr, 128 * toff + p + 64 * dr] = F8(-448.0)
    eye = np.zeros((128, 2, 128), dtype=F8)
    for dr in range(2):
        eye[p, dr, p + 64 * dr] = F8(64.0)

    in_maps = []
    for c in range(M):
        jsl = slice(c * JC, (c + 1) * JC)
        # st8[pk, jt, kh, dr, j] = r2_8[c*JC + 128*jt + j, kh*256 + dr*128 + pk]
        st8 = np.ascontiguousarray(
            r2_8[jsl].reshape(NJT, 128, 2, 2, 128).transpose(4, 0, 2, 3, 1)
        )
        # i-rotation: logical i -> physical (i + c*JC) % B
        iperm = (np.arange(B) + c * JC) % B
        xr = x8[iperm]                                         # [B, D] rotated
        # xt8[pk, ih, kh, dr, i] = xr[ih*IH + i, kh*256 + dr*128 + pk]
        xt8 = np.ascontiguousarray(
            xr.reshape(2, IH, 2, 2, 128).transpose(4, 0, 2, 3, 1)
        )
        rsqT = np.ascontiguousarray(
            (-rsq[jsl].reshape(NJT, 128).T).astype(np.float32)
        )
        in_maps.append(
            {"st8": st8, "xt8": xt8, "rsqT": rsqT, "mask": maskr, "eye": eye}
        )
    return in_maps


def _combine(outs, feature_ts, feature_image1, feature_image2):
    ts = np.asarray(feature_ts, dtype=np.float64)
    im1 = np.asarray(feature_image1, dtype=np.float64)
    im2 = np.asarray(feature_image2, dtype=np.float64)
    l1 = np.sqrt(((ts - im1) ** 2).sum(1))
    l2 = np.sqrt(((ts - im2) ** 2).sum(1))
    d12 = np.sqrt(((im1 - im2 + 1e-6) ** 2).sum(1))
    xsq = (ts ** 2).sum(1)

    smax = np.full(B, -np.inf)
    for c, (oD, oA) in enumerate(outs):
        s = np.maximum(
            np.asarray(oD, dtype=np.float64).max(axis=0),
            np.asarray(oA, dtype=np.float64).max(axis=0),
        )                                                       # [B] logical i
        phys = (np.arange(B) + c * JC) % B
        np.maximum.at(smax, phys, s)
    negsq = xsq - smax
    neg = np.sqrt(np.maximum(negsq, 0.0))
    trip = np.maximum(l1 + l2 + d12 - neg + 0.1, 0.0) + np.maximum(l1, l2)
    return np.float32(trip.sum() / B)


def kernel(feature_ts, feature_image1, feature_image2, _trace=False):
    global _NC_CACHE, LAST_RESULTS
    if _NC_CACHE is None:
        _NC_CACHE = _build_nc()
    if _trace:
        _install_ntff_hook()
    in_maps = _host_inputs(feature_ts, feature_image1, feature_image2)
    res = run_bass_kernel_spmd(_NC_CACHE, in_maps, list(range(M)), trace=_trace)
    LAST_RESULTS = res
    return _combine(
        [(res.results[c]["oD"], res.results[c]["oA"]) for c in range(M)],
        feature_ts, feature_image1, feature_image2,
    )



# revision 6
# speedup vs baseline: 1.0462x; 1.0462x over previous
"""DRCLoss kernel v4 for 8 Trainium2 NeuronCores (Bass/Tile, SPMD).

Math: loss = mean_i[ relu(l1_i + l2_i + d12_i - neg_i + 0.1) + max(l1_i, l2_i) ]
  where neg_i = min over non-self columns of cdist(ts, [ts; im1; im2])[i, :].

v4 strategy (column-sharded: each core owns 1536 of the 12288 columns j,
all 4096 rows i):
  - Device computes s[j, i] = 2*r_j.x_i - rsq[j] with output partitions = j.
    Only the masked column-max of s survives on device (negsq = xsq - max);
    l1/l2/d12/xsq are exact host math.
  - i is processed in 4 quarters of 1024; per (iq, jt) quad the PSUM block is
    [128, 1024] (2 banks), filled by 4 DR matmuls (2 kh x 2 ib of 512).
    Output per quarter streams out early, so the tail only exposes the last
    quarter's fold + a 256 KB DMA.
  - Single fp16 accumulator per quarter; fold paths balanced across engines:
      jt==0 : ScalarE activation copy psum+(-rsq) -> acc   (no max needed)
      jt in D_SET: DVE scalar_tensor_tensor acc = max(acc, psum + (-rsq))
      else  : ScalarE psum+(-rsq) -> fp16 tmp, DVE tensor_max fold
  - Self-column exclusion: per-core i-rotation puts the self diagonal of
    j-tile jt at rows jt*128+p; one tiny N=128 plain-fp8 matmul
    (eye*64 x eye*(-448)) adds -28672 there in PSUM.
  - Input DMA launches are spread across engines (sync/scalar/vector/gpsimd)
    and chunked so the first matmul only waits for st[jt0..5] + xt[iq0];
    a garbage warmup burst ramps the PE HAM clock during the DMA wait.
  - Host finishes: negsq = xsq - max over cores/partitions, sqrt/relu/mean
    in float64.
"""

import sys

if "/opt/trn_rl_repo" not in sys.path:
    sys.path.insert(0, "/opt/trn_rl_repo")

from contextlib import ExitStack

import ml_dtypes
import numpy as np

import concourse.bass as bass
import concourse.tile as tile
from concourse import mybir
from concourse.bass_utils import run_bass_kernel_spmd

BF16 = ml_dtypes.bfloat16
F8 = ml_dtypes.float8_e4m3

B = 4096          # rows
D = 512           # feature dim
M = 8             # cores
JC = 3 * B // M   # columns per core (1536)
NJT = JC // 128   # j-tiles per core (12)
NIQ = 4           # i quarters
IW = B // NIQ     # rows per quarter (1024)

# "dr" = hardware DoubleRow; "dri" = DoubleRowSwInterleave (host-interleaved
# weights so LDWEIGHTS streams contiguously)
PERF_MODE = "dr"

# j-tiles folded via the DVE scalar_tensor_tensor path (rest: ScalarE copy +
# DVE fp16 max). jt==0 is always the ScalarE direct-copy path.
D_SET = (3, 5, 8, 11)

LAST_RESULTS = None

_NC_CACHE = None
_NC_CACHE_MODE = None


def _install_ntff_hook():
    """Provide antenv.axon_hooks (missing in this image) so trace=True can
    capture NTFF profiles through libaxon_pjrt.so."""
    try:
        import antenv.axon_hooks  # noqa: F401

        return
    except ImportError:
        pass
    try:
        import types

        import antenv
        from trn_agent_boot.trn_boot import _ntff_profile_via_ctypes

        mod = types.ModuleType("antenv.axon_hooks")
        mod._hook = None

        def set_axon_ntff_profile_hook(h):
            mod._hook = h

        def get_axon_ntff_profile_hook():
            return mod._hook

        mod.set_axon_ntff_profile_hook = set_axon_ntff_profile_hook
        mod.get_axon_ntff_profile_hook = get_axon_ntff_profile_hook
        sys.modules["antenv.axon_hooks"] = mod
        antenv.axon_hooks = mod
        hook = _ntff_profile_via_ctypes("/opt/axon/libaxon_pjrt.so")
        if hook is not None:
            mod._hook = hook
    except Exception:
        pass


def _split_multi_waits(nc):
    """This walrus build allows only ONE embedded sync wait per instruction.
    Hoist extra waits onto standalone EventSemaphore instructions inserted
    just before the owner (same engine, so program order is preserved)."""
    import bass_rust

    ctr = 0
    for blk in nc.m.functions[0].blocks:
        il = blk.instructions
        new = []
        for inst in il:
            si = getattr(inst, "sync_info", None)
            waits = list(si.on_wait) if si is not None else []
            if len(waits) > 1:
                for w in waits[:-1]:
                    ev = bass_rust.InstEventSemaphore(name=f"wsplit_{ctr}")
                    ctr += 1
                    ev.engine = inst.engine
                    ev.sync_info = bass_rust.SyncInfo(on_wait=[w], on_update=[])
                    new.append(ev)
                inst.sync_info = bass_rust.SyncInfo(
                    on_wait=[waits[-1]], on_update=list(si.on_update)
                )
            new.append(inst)
        il[:] = new


def _build_nc(perf_mode):
    nc = bass.Bass()
    f32 = mybir.dt.float32
    f16 = mybir.dt.float16
    f8 = mybir.dt.float8e4
    if perf_mode == "dri":
        PM = mybir.MatmulPerfMode.DoubleRowSwInterleave
    else:
        PM = mybir.MatmulPerfMode.DoubleRow
    add = mybir.AluOpType.add
    mx = mybir.AluOpType.max

    # stationary R-chunks [pk, jt, kh, dr, j] (dr x j byte order per perf mode)
    st_d = nc.dram_tensor("st8", [128, NJT, 2, 2, 128], f8, kind="ExternalInput")
    # moving X^T, i-rotated per core, iq-major [pk, iq, kh, dr, i]
    xt_d = nc.dram_tensor("xt8", [128, NIQ, 2, 2, IW], f8, kind="ExternalInput")
    rsqT_d = nc.dram_tensor("rsqT", [128, NJT], f32, kind="ExternalInput")
    # eyes[:, 0] = I*64, eyes[:, 1] = I*(-448)
    eyes_d = nc.dram_tensor("eyes", [128, 2, 128], f8, kind="ExternalInput")
    oA_d = nc.dram_tensor("oA", [128, B], f16, kind="ExternalOutput")

    with ExitStack() as ctx:
        tc = ctx.enter_context(tile.TileContext(nc))
        const = ctx.enter_context(tc.tile_pool(name="const", bufs=1))
        hpp = ctx.enter_context(tc.tile_pool(name="hp", bufs=3))
        accp = ctx.enter_context(tc.tile_pool(name="acc", bufs=2))
        psump = ctx.enter_context(tc.tile_pool(name="psum", bufs=3, space="PSUM"))

        def dummy_mm(lhs_ap, rhs_ap):
            pw = psump.tile([128, IW], f32, tag="q", name="pdum")
            nc.tensor.matmul(pw[: lhs_ap.shape[-1], : rhs_ap.shape[-1]],
                             lhs_ap, rhs_ap, start=True, stop=True)

        # --- input DMA launches, spread across engines so the descriptors
        # generate in parallel ---
        garb = const.tile([128, 512], f8, tag="garb")
        nc.vector.memset(garb, 0.0)
        eyes = const.tile([128, 2, 128], f8, tag="eyes")
        rsqT = const.tile([128, NJT], f32, tag="rsqT")
        xt = const.tile([128, NIQ, 2, 2, IW], f8, tag="xt")
        st = const.tile([128, NJT, 2, 2, 128], f8, tag="st")

        nc.sync.dma_start(out=xt[:, 0, 0], in_=xt_d[:, 0, 0])
        nc.sync.dma_start(out=xt[:, 0, 1], in_=xt_d[:, 0, 1])
        nc.scalar.dma_start(out=st[:, 0:6], in_=st_d[:, 0:6])
        nc.scalar.dma_start(out=eyes, in_=eyes_d[:, :])
        nc.scalar.dma_start(out=rsqT, in_=rsqT_d[:, :])
        nc.scalar.dma_start(out=st[:, 6:NJT], in_=st_d[:, 6:NJT])
        nc.gpsimd.dma_start(out=xt[:, 1], in_=xt_d[:, 1])
        nc.gpsimd.dma_start(out=xt[:, 2], in_=xt_d[:, 2])
        nc.gpsimd.dma_start(out=xt[:, 3], in_=xt_d[:, 3])

        # warmup burst on garbage data: ramps the PE HAM clock while the
        # input DMAs land (PE must stay busy ~3.4us to reach 2.4 GHz)
        for _ in range(10):
            pw = psump.tile([128, IW], f32, tag="q", name="pwarm")
            nc.tensor.matmul(pw[:, 0:512], garb[:, 0:128], garb[:, 0:512],
                             start=True, stop=True)

        # absorb DMA-completion waits on the consuming engines early
        dummy_mm(st[:, 0, 0, 0, 0:4], st[:, 0, 0, 0, 0:8])
        dummy_mm(xt[:, 0, 0, 0, 0:4], xt[:, 0, 0, 0, 0:8])
        dummy_mm(eyes[:, 0, 0:4], eyes[:, 0, 0:8])
        vabs = const.tile([128, 1], f32, tag="vabs")
        nc.vector.tensor_copy(vabs, rsqT[:, 0:1])
        sabs = const.tile([128, 1], f32, tag="sabs")
        nc.scalar.copy(sabs, rsqT[:, 0:1])

        for iq in range(NIQ):
            acc = accp.tile([128, IW], f16, tag="acc")
            for jt in range(NJT):
                q = psump.tile([128, IW], f32, tag="q", name="q")
                # diagonal (self) block of this j-tile, if it lands in iq
                moff = jt * 128 - iq * IW
                has_mask = 0 <= moff < IW
                for kh in range(2):
                    for ib in range(2):
                        nc.tensor.matmul(
                            q[:, ib * 512 : (ib + 1) * 512],
                            st[:, jt, kh],
                            xt[:, iq, kh, :, ib * 512 : (ib + 1) * 512],
                            start=(kh == 0),
                            stop=(kh == 1 and not has_mask),
                            perf_mode=PM,
                            skip_group_check=has_mask,
                        )
                if has_mask:
                    nc.tensor.matmul(
                        q[:, moff : moff + 128],
                        eyes[:, 0],
                        eyes[:, 1],
                        start=False,
                        stop=True,
                        skip_group_check=True,
                    )
                if jt == 0:
                    nc.scalar.add(acc, q, rsqT[:, 0:1])
                elif jt in D_SET:
                    nc.vector.scalar_tensor_tensor(
                        out=acc, in0=q, scalar=rsqT[:, jt : jt + 1], in1=acc,
                        op0=add, op1=mx,
                    )
                else:
                    hp = hpp.tile([128, IW], f16, tag="hp")
                    nc.scalar.add(hp, q, rsqT[:, jt : jt + 1])
                    nc.vector.tensor_max(acc, acc, hp)
            nc.gpsimd.dma_start(out=oA_d[:, iq * IW : (iq + 1) * IW], in_=acc)

    _split_multi_waits(nc)
    return nc


def _host_inputs(feature_ts, feature_image1, feature_image2):
    ts = np.ascontiguousarray(feature_ts, dtype=np.float32)
    im1 = np.ascontiguousarray(feature_image1, dtype=np.float32)
    im2 = np.ascontiguousarray(feature_image2, dtype=np.float32)

    R = np.concatenate([ts, im1, im2], 0)                      # [3B, D]
    rsq = (R.astype(np.float64) ** 2).sum(1)                   # [3B]
    x8 = ts.astype(F8)                                         # [B, D]
    r2_8 = (2.0 * R.astype(F8).astype(np.float32)).astype(F8)  # exact doubling

    eyes = np.zeros((128, 2, 128), dtype=F8)
    p = np.arange(128)
    eyes[p, 0, p] = F8(64.0)
    eyes[p, 1, p] = F8(-448.0)

    in_maps = []
    for c in range(M):
        jsl = slice(c * JC, (c + 1) * JC)
        # st5[pk, jt, kh, dr, j] = r2_8[c*JC + 128*jt + j, kh*256 + dr*128 + pk]
        st5 = r2_8[jsl].reshape(NJT, 128, 2, 2, 128).transpose(4, 0, 2, 3, 1)
        if PERF_MODE == "dri":
            # SwInterleave layout: per weight load the 256 bytes are
            # [A127, B127, A126, B126, ..., A0, B0] (A = dr0, B = dr1,
            # columns reversed)
            sti = np.empty((128, NJT, 2, 256), dtype=F8)
            sti[..., 0::2] = st5[..., 0, ::-1]
            sti[..., 1::2] = st5[..., 1, ::-1]
            sti = sti.reshape(128, NJT, 2, 2, 128)
        else:
            sti = np.ascontiguousarray(st5)
        # i-rotation: physical row ip holds logical row (ip + c*JC) % B, so
        # the self diagonal of j-tile jt sits at rows jt*128 + p on all cores
        iperm = (np.arange(B) + c * JC) % B
        xr = x8[iperm]                                         # [B, D] rotated
        # xt8[pk, iq, kh, dr, i] = xr[iq*IW + i, kh*256 + dr*128 + pk]
        xt8 = np.ascontiguousarray(
            xr.reshape(NIQ, IW, 2, 2, 128).transpose(4, 0, 2, 3, 1)
        )
        rsqT = np.ascontiguousarray(
            (-rsq[jsl].reshape(NJT, 128).T).astype(np.float32)
        )
        in_maps.append({"st8": sti, "xt8": xt8, "rsqT": rsqT, "eyes": eyes})
    return in_maps


def _combine(outs, feature_ts, feature_image1, feature_image2):
    ts = np.asarray(feature_ts, dtype=np.float64)
    im1 = np.asarray(feature_image1, dtype=np.float64)
    im2 = np.asarray(feature_image2, dtype=np.float64)
    l1 = np.sqrt(((ts - im1) ** 2).sum(1))
    l2 = np.sqrt(((ts - im2) ** 2).sum(1))
    d12 = np.sqrt(((im1 - im2 + 1e-6) ** 2).sum(1))
    xsq = (ts ** 2).sum(1)

    smax = np.full(B, -np.inf)
    for c, oA in enumerate(outs):
        s = np.asarray(oA, dtype=np.float64).max(axis=0)        # [B] physical i
        phys = (np.arange(B) + c * JC) % B
        np.maximum.at(smax, phys, s)
    negsq = xsq - smax
    neg = np.sqrt(np.maximum(negsq, 0.0))
    trip = np.maximum(l1 + l2 + d12 - neg + 0.1, 0.0) + np.maximum(l1, l2)
    return np.float32(trip.sum() / B)


def kernel(feature_ts, feature_image1, feature_image2, _trace=False):
    global _NC_CACHE, _NC_CACHE_MODE, LAST_RESULTS
    if _NC_CACHE is None or _NC_CACHE_MODE != PERF_MODE:
        _NC_CACHE = _build_nc(PERF_MODE)
        _NC_CACHE_MODE = PERF_MODE
    if _trace:
        _install_ntff_hook()
    in_maps = _host_inputs(feature_ts, feature_image1, feature_image2)
    res = run_bass_kernel_spmd(_NC_CACHE, in_maps, list(range(M)), trace=_trace)
    LAST_RESULTS = res
    return _combine(
        [res.results[c]["oA"] for c in range(M)],
        feature_ts, feature_image1, feature_image2,
    )


# revision 10
# speedup vs baseline: 1.1574x; 1.1063x over previous
"""DRCLoss kernel v4 for 8 Trainium2 NeuronCores (Bass/Tile, SPMD).

Math: loss = mean_i[ relu(l1_i + l2_i + d12_i - neg_i + 0.1) + max(l1_i, l2_i) ]
  where neg_i = min over non-self columns of cdist(ts, [ts; im1; im2])[i, :].

v4 strategy (column-sharded: each core owns 1536 of the 12288 columns j,
all 4096 rows i):
  - Device computes s[j, i] = 2*r_j.x_i - rsq[j] with output partitions = j.
    Only the masked column-max of s survives on device (negsq = xsq - max);
    l1/l2/d12/xsq are exact host math.
  - i is processed in 4 quarters of 1024; per (iq, jt) quad the PSUM block is
    [128, 1024] (2 banks), filled by 4 DR matmuls (2 kh x 2 ib of 512).
    Output per quarter streams out early, so the tail only exposes the last
    quarter's fold + a 256 KB DMA.
  - Single fp16 accumulator per quarter; fold paths balanced across engines:
      jt==0 : ScalarE activation copy psum+(-rsq) -> acc   (no max needed)
      jt in D_SET: DVE scalar_tensor_tensor acc = max(acc, psum + (-rsq))
      else  : ScalarE psum+(-rsq) -> fp16 tmp, DVE tensor_max fold
  - Self-column exclusion: per-core i-rotation puts the self diagonal of
    j-tile jt at rows jt*128+p; one tiny N=128 plain-fp8 matmul
    (eye*64 x eye*(-448)) adds -28672 there in PSUM.
  - Input DMA launches are spread across engines (sync/scalar/vector/gpsimd)
    and chunked so the first matmul only waits for st[jt0..5] + xt[iq0];
    a garbage warmup burst ramps the PE HAM clock during the DMA wait.
  - Host finishes: negsq = xsq - max over cores/partitions, sqrt/relu/mean
    in float64.
"""

import sys

if "/opt/trn_rl_repo" not in sys.path:
    sys.path.insert(0, "/opt/trn_rl_repo")

from contextlib import ExitStack

import ml_dtypes
import numpy as np

import concourse.bass as bass
import concourse.tile as tile
from concourse import mybir
from concourse.bass_utils import run_bass_kernel_spmd

BF16 = ml_dtypes.bfloat16
F8 = ml_dtypes.float8_e4m3

B = 4096          # rows
D = 512           # feature dim
M = 8             # cores
JC = 3 * B // M   # columns per core (1536)
NJT = JC // 128   # j-tiles per core (12)
NIQ = 4           # i quarters
IW = B // NIQ     # rows per quarter (1024)

# "dr" = hardware DoubleRow; "dri" = DoubleRowSwInterleave (host-interleaved
# weights so LDWEIGHTS streams contiguously)
PERF_MODE = "dr"

# j-tiles folded via the DVE scalar_tensor_tensor path (rest: ScalarE copy +
# DVE fp16 max). jt==0 is always the ScalarE direct-copy path.
D_SET = (3, 8, 11)

LAST_RESULTS = None

_NC_CACHE = None
_NC_CACHE_MODE = None


def _install_ntff_hook():
    """Provide antenv.axon_hooks (missing in this image) so trace=True can
    capture NTFF profiles through libaxon_pjrt.so."""
    try:
        import antenv.axon_hooks  # noqa: F401

        return
    except ImportError:
        pass
    try:
        import types

        import antenv
        from trn_agent_boot.trn_boot import _ntff_profile_via_ctypes

        mod = types.ModuleType("antenv.axon_hooks")
        mod._hook = None

        def set_axon_ntff_profile_hook(h):
            mod._hook = h

        def get_axon_ntff_profile_hook():
            return mod._hook

        mod.set_axon_ntff_profile_hook = set_axon_ntff_profile_hook
        mod.get_axon_ntff_profile_hook = get_axon_ntff_profile_hook
        sys.modules["antenv.axon_hooks"] = mod
        antenv.axon_hooks = mod
        hook = _ntff_profile_via_ctypes("/opt/axon/libaxon_pjrt.so")
        if hook is not None:
            mod._hook = hook
    except Exception:
        pass


def _split_multi_waits(nc):
    """This walrus build allows only ONE embedded sync wait per instruction.
    Hoist extra waits onto standalone EventSemaphore instructions inserted
    just before the owner (same engine, so program order is preserved)."""
    import bass_rust

    ctr = 0
    for blk in nc.m.functions[0].blocks:
        il = blk.instructions
        new = []
        for inst in il:
            si = getattr(inst, "sync_info", None)
            waits = list(si.on_wait) if si is not None else []
            if len(waits) > 1:
                for w in waits[:-1]:
                    ev = bass_rust.InstEventSemaphore(name=f"wsplit_{ctr}")
                    ctr += 1
                    ev.engine = inst.engine
                    ev.sync_info = bass_rust.SyncInfo(on_wait=[w], on_update=[])
                    new.append(ev)
                inst.sync_info = bass_rust.SyncInfo(
                    on_wait=[waits[-1]], on_update=list(si.on_update)
                )
            new.append(inst)
        il[:] = new


def _build_nc(perf_mode):
    nc = bass.Bass()
    f32 = mybir.dt.float32
    f16 = mybir.dt.float16
    f8 = mybir.dt.float8e4
    if perf_mode == "dri":
        PM = mybir.MatmulPerfMode.DoubleRowSwInterleave
    else:
        PM = mybir.MatmulPerfMode.DoubleRow
    add = mybir.AluOpType.add
    mx = mybir.AluOpType.max

    # stationary R-chunks [pk, jt, kh, dr, j] (dr x j byte order per perf mode)
    st_d = nc.dram_tensor("st8", [128, NJT, 2, 2, 128], f8, kind="ExternalInput")
    # moving X^T, i-rotated per core, iq-major [pk, iq, kh, dr, i]
    xt_d = nc.dram_tensor("xt8", [128, NIQ, 2, 2, IW], f8, kind="ExternalInput")
    rsqT_d = nc.dram_tensor("rsqT", [128, NJT], f32, kind="ExternalInput")
    # eyes[:, 0] = I*64, eyes[:, 1] = I*(-448)
    eyes_d = nc.dram_tensor("eyes", [128, 2, 128], f8, kind="ExternalInput")
    oA_d = nc.dram_tensor("oA", [128, B], f16, kind="ExternalOutput")

    with ExitStack() as ctx:
        tc = ctx.enter_context(tile.TileContext(nc))
        const = ctx.enter_context(tc.tile_pool(name="const", bufs=1))
        hpp = ctx.enter_context(tc.tile_pool(name="hp", bufs=3))
        accp = ctx.enter_context(tc.tile_pool(name="acc", bufs=2))
        psump = ctx.enter_context(tc.tile_pool(name="psum", bufs=4, space="PSUM"))

        def dummy_mm(lhs_ap, rhs_ap):
            pw = psump.tile([128, IW], f32, tag="q", name="pdum")
            nc.tensor.matmul(pw[: lhs_ap.shape[-1], : rhs_ap.shape[-1]],
                             lhs_ap, rhs_ap, start=True, stop=True)

        # --- input DMA launches, spread across engines so the descriptors
        # generate in parallel ---
        garb = const.tile([128, 512], f8, tag="garb")
        nc.vector.memset(garb, 0.0)
        eyes = const.tile([128, 2, 128], f8, tag="eyes")
        rsqT = const.tile([128, NJT], f32, tag="rsqT")
        xt = const.tile([128, NIQ, 2, 2, IW], f8, tag="xt")
        st = const.tile([128, NJT, 2, 2, 128], f8, tag="st")

        nc.sync.dma_start(out=xt[:, 0, 0], in_=xt_d[:, 0, 0])
        nc.sync.dma_start(out=xt[:, 0, 1], in_=xt_d[:, 0, 1])
        nc.scalar.dma_start(out=eyes, in_=eyes_d[:, :])
        nc.scalar.dma_start(out=rsqT, in_=rsqT_d[:, :])
        nc.scalar.dma_start(out=st[:, 0:6], in_=st_d[:, 0:6])
        nc.scalar.dma_start(out=st[:, 6:NJT], in_=st_d[:, 6:NJT])
        nc.gpsimd.dma_start(out=xt[:, 1], in_=xt_d[:, 1])
        nc.gpsimd.dma_start(out=xt[:, 2], in_=xt_d[:, 2])
        nc.gpsimd.dma_start(out=xt[:, 3], in_=xt_d[:, 3])

        # warmup burst on garbage data: ramps the PE HAM clock while the
        # input DMAs land (PE must stay busy ~3.4us to reach 2.4 GHz)
        for _ in range(10):
            pw = psump.tile([128, IW], f32, tag="q", name="pwarm")
            nc.tensor.matmul(pw[:, 0:512], garb[:, 0:128], garb[:, 0:512],
                             start=True, stop=True)

        # absorb DMA-completion waits on the consuming engines early
        dummy_mm(st[:, 0, 0, 0, 0:4], st[:, 0, 0, 0, 0:8])
        dummy_mm(xt[:, 0, 0, 0, 0:4], xt[:, 0, 0, 0, 0:8])
        dummy_mm(eyes[:, 0, 0:4], eyes[:, 0, 0:8])
        vabs = const.tile([128, 1], f32, tag="vabs")
        nc.vector.tensor_copy(vabs, rsqT[:, 0:1])
        sabs = const.tile([128, 1], f32, tag="sabs")
        nc.scalar.copy(sabs, rsqT[:, 0:1])

        for iq in range(NIQ):
            acc = accp.tile([128, IW], f16, tag="acc")
            for jt in range(NJT):
                q = psump.tile([128, IW], f32, tag="q", name="q")
                # diagonal (self) block of this j-tile, if it lands in iq
                moff = jt * 128 - iq * IW
                has_mask = 0 <= moff < IW
                for kh in range(2):
                    for ib in range(2):
                        nc.tensor.matmul(
                            q[:, ib * 512 : (ib + 1) * 512],
                            st[:, jt, kh],
                            xt[:, iq, kh, :, ib * 512 : (ib + 1) * 512],
                            start=(kh == 0),
                            stop=(kh == 1 and not has_mask),
                            perf_mode=PM,
                            skip_group_check=has_mask,
                        )
                if has_mask:
                    nc.tensor.matmul(
                        q[:, moff : moff + 128],
                        eyes[:, 0],
                        eyes[:, 1],
                        start=False,
                        stop=True,
                        skip_group_check=True,
                    )
                if jt == 0:
                    nc.scalar.add(acc, q, rsqT[:, 0:1])
                elif jt in D_SET:
                    nc.vector.scalar_tensor_tensor(
                        out=acc, in0=q, scalar=rsqT[:, jt : jt + 1], in1=acc,
                        op0=add, op1=mx,
                    )
                else:
                    hp = hpp.tile([128, IW], f16, tag="hp")
                    nc.scalar.add(hp, q, rsqT[:, jt : jt + 1])
                    nc.vector.tensor_max(acc, acc, hp)
            nc.sync.dma_start(out=oA_d[:, iq * IW : (iq + 1) * IW], in_=acc)

    _split_multi_waits(nc)
    return nc


def _host_inputs(feature_ts, feature_image1, feature_image2):
    ts = np.ascontiguousarray(feature_ts, dtype=np.float32)
    im1 = np.ascontiguousarray(feature_image1, dtype=np.float32)
    im2 = np.ascontiguousarray(feature_image2, dtype=np.float32)

    R = np.concatenate([ts, im1, im2], 0)                      # [3B, D]
    rsq = (R.astype(np.float64) ** 2).sum(1)                   # [3B]
    x8 = ts.astype(F8)                                         # [B, D]
    r2_8 = (2.0 * R.astype(F8).astype(np.float32)).astype(F8)  # exact doubling

    eyes = np.zeros((128, 2, 128), dtype=F8)
    p = np.arange(128)
    eyes[p, 0, p] = F8(64.0)
    eyes[p, 1, p] = F8(-448.0)

    in_maps = []
    for c in range(M):
        jsl = slice(c * JC, (c + 1) * JC)
        # st5[pk, jt, kh, dr, j] = r2_8[c*JC + 128*jt + j, kh*256 + dr*128 + pk]
        st5 = r2_8[jsl].reshape(NJT, 128, 2, 2, 128).transpose(4, 0, 2, 3, 1)
        if PERF_MODE == "dri":
            # SwInterleave layout: per weight load the 256 bytes are
            # [A127, B127, A126, B126, ..., A0, B0] (A = dr0, B = dr1,
            # columns reversed)
            sti = np.empty((128, NJT, 2, 256), dtype=F8)
            sti[..., 0::2] = st5[..., 0, ::-1]
            sti[..., 1::2] = st5[..., 1, ::-1]
            sti = sti.reshape(128, NJT, 2, 2, 128)
        else:
            sti = np.ascontiguousarray(st5)
        # i-rotation: physical row ip holds logical row (ip + c*JC) % B, so
        # the self diagonal of j-tile jt sits at rows jt*128 + p on all cores
        iperm = (np.arange(B) + c * JC) % B
        xr = x8[iperm]                                         # [B, D] rotated
        # xt8[pk, iq, kh, dr, i] = xr[iq*IW + i, kh*256 + dr*128 + pk]
        xt8 = np.ascontiguousarray(
            xr.reshape(NIQ, IW, 2, 2, 128).transpose(4, 0, 2, 3, 1)
        )
        rsqT = np.ascontiguousarray(
            (-rsq[jsl].reshape(NJT, 128).T).astype(np.float32)
        )
        in_maps.append({"st8": sti, "xt8": xt8, "rsqT": rsqT, "eyes": eyes})
    return in_maps


def _combine(outs, feature_ts, feature_image1, feature_image2):
    ts = np.asarray(feature_ts, dtype=np.float64)
    im1 = np.asarray(feature_image1, dtype=np.float64)
    im2 = np.asarray(feature_image2, dtype=np.float64)
    l1 = np.sqrt(((ts - im1) ** 2).sum(1))
    l2 = np.sqrt(((ts - im2) ** 2).sum(1))
    d12 = np.sqrt(((im1 - im2 + 1e-6) ** 2).sum(1))
    xsq = (ts ** 2).sum(1)

    smax = np.full(B, -np.inf)
    for c, oA in enumerate(outs):
        s = np.asarray(oA, dtype=np.float64).max(axis=0)        # [B] physical i
        phys = (np.arange(B) + c * JC) % B
        np.maximum.at(smax, phys, s)
    negsq = xsq - smax
    neg = np.sqrt(np.maximum(negsq, 0.0))
    trip = np.maximum(l1 + l2 + d12 - neg + 0.1, 0.0) + np.maximum(l1, l2)
    return np.float32(trip.sum() / B)


def kernel(feature_ts, feature_image1, feature_image2, _trace=False):
    global _NC_CACHE, _NC_CACHE_MODE, LAST_RESULTS
    if _NC_CACHE is None or _NC_CACHE_MODE != PERF_MODE:
        _NC_CACHE = _build_nc(PERF_MODE)
        _NC_CACHE_MODE = PERF_MODE
    if _trace:
        _install_ntff_hook()
    in_maps = _host_inputs(feature_ts, feature_image1, feature_image2)
    res = run_bass_kernel_spmd(_NC_CACHE, in_maps, list(range(M)), trace=_trace)
    LAST_RESULTS = res
    return _combine(
        [res.results[c]["oA"] for c in range(M)],
        feature_ts, feature_image1, feature_image2,
    )


# revision 15
# speedup vs baseline: 1.2157x; 1.0504x over previous
"""DRCLoss kernel v4 for 8 Trainium2 NeuronCores (Bass/Tile, SPMD).

Math: loss = mean_i[ relu(l1_i + l2_i + d12_i - neg_i + 0.1) + max(l1_i, l2_i) ]
  where neg_i = min over non-self columns of cdist(ts, [ts; im1; im2])[i, :].

v4 strategy (column-sharded: each core owns 1536 of the 12288 columns j,
all 4096 rows i):
  - Device computes s[j, i] = 2*r_j.x_i - rsq[j] with output partitions = j.
    Only the masked column-max of s survives on device (negsq = xsq - max);
    l1/l2/d12/xsq are exact host math.
  - i is processed in 4 quarters of 1024; per (iq, jt) quad the PSUM block is
    [128, 1024] (2 banks), filled by 4 DR matmuls (2 kh x 2 ib of 512).
    Output per quarter streams out early, so the tail only exposes the last
    quarter's fold + a 256 KB DMA.
  - Single fp16 accumulator per quarter; fold paths balanced across engines:
      jt==0 : ScalarE activation copy psum+(-rsq) -> acc   (no max needed)
      jt in D_SET: DVE scalar_tensor_tensor acc = max(acc, psum + (-rsq))
      else  : ScalarE psum+(-rsq) -> fp16 tmp, DVE tensor_max fold
  - Self-column exclusion: per-core i-rotation puts the self diagonal of
    j-tile jt at rows jt*128+p; one tiny N=128 plain-fp8 matmul
    (eye*64 x eye*(-448)) adds -28672 there in PSUM.
  - Input DMA launches are spread across engines (sync/scalar/vector/gpsimd)
    and chunked so the first matmul only waits for st[jt0..5] + xt[iq0];
    a garbage warmup burst ramps the PE HAM clock during the DMA wait.
  - Host finishes: negsq = xsq - max over cores/partitions, sqrt/relu/mean
    in float64.
"""

import sys

if "/opt/trn_rl_repo" not in sys.path:
    sys.path.insert(0, "/opt/trn_rl_repo")

from contextlib import ExitStack

import ml_dtypes
import numpy as np

import concourse.bass as bass
import concourse.tile as tile
from concourse import mybir
from concourse.bass_utils import run_bass_kernel_spmd

BF16 = ml_dtypes.bfloat16
F8 = ml_dtypes.float8_e4m3

B = 4096          # rows
D = 512           # feature dim
M = 8             # cores
JC = 3 * B // M   # columns per core (1536)
NJT = JC // 128   # j-tiles per core (12)
NIQ = 4           # i quarters
IW = B // NIQ     # rows per quarter (1024)

# "dr" = hardware DoubleRow; "dri" = DoubleRowSwInterleave (host-interleaved
# weights so LDWEIGHTS streams contiguously)
PERF_MODE = "dr"

# j-tiles folded via the DVE scalar_tensor_tensor path (rest: ScalarE copy +
# DVE fp16 max). jt==0 is always the ScalarE direct-copy path.
D_SET = (3, 8, 11)
# in the last i-quarter, fold jt 11 right after jt 0 so the final exposed
# fold is the cheap A-path one that overlaps the closing matmuls
JT_TAIL_ORDER = (0, 11, 1, 2, 3, 4, 5, 6, 7, 8, 9, 10)

LAST_RESULTS = None

_NC_CACHE = None
_NC_CACHE_MODE = None


def _install_ntff_hook():
    """Provide antenv.axon_hooks (missing in this image) so trace=True can
    capture NTFF profiles through libaxon_pjrt.so."""
    try:
        import antenv.axon_hooks  # noqa: F401

        return
    except ImportError:
        pass
    try:
        import types

        import antenv
        from trn_agent_boot.trn_boot import _ntff_profile_via_ctypes

        mod = types.ModuleType("antenv.axon_hooks")
        mod._hook = None

        def set_axon_ntff_profile_hook(h):
            mod._hook = h

        def get_axon_ntff_profile_hook():
            return mod._hook

        mod.set_axon_ntff_profile_hook = set_axon_ntff_profile_hook
        mod.get_axon_ntff_profile_hook = get_axon_ntff_profile_hook
        sys.modules["antenv.axon_hooks"] = mod
        antenv.axon_hooks = mod
        hook = _ntff_profile_via_ctypes("/opt/axon/libaxon_pjrt.so")
        if hook is not None:
            mod._hook = hook
    except Exception:
        pass


def _split_multi_waits(nc):
    """This walrus build allows only ONE embedded sync wait per instruction.
    Hoist extra waits onto standalone EventSemaphore instructions inserted
    just before the owner (same engine, so program order is preserved)."""
    import bass_rust

    ctr = 0
    for blk in nc.m.functions[0].blocks:
        il = blk.instructions
        new = []
        for inst in il:
            si = getattr(inst, "sync_info", None)
            waits = list(si.on_wait) if si is not None else []
            if len(waits) > 1:
                for w in waits[:-1]:
                    ev = bass_rust.InstEventSemaphore(name=f"wsplit_{ctr}")
                    ctr += 1
                    ev.engine = inst.engine
                    ev.sync_info = bass_rust.SyncInfo(on_wait=[w], on_update=[])
                    new.append(ev)
                inst.sync_info = bass_rust.SyncInfo(
                    on_wait=[waits[-1]], on_update=list(si.on_update)
                )
            new.append(inst)
        il[:] = new


def _build_nc(perf_mode):
    nc = bass.Bass()
    f32 = mybir.dt.float32
    f16 = mybir.dt.float16
    f8 = mybir.dt.float8e4
    if perf_mode == "dri":
        PM = mybir.MatmulPerfMode.DoubleRowSwInterleave
    else:
        PM = mybir.MatmulPerfMode.DoubleRow
    add = mybir.AluOpType.add
    mx = mybir.AluOpType.max

    # stationary R-chunks [pk, jt, kh, dr, j] (dr x j byte order per perf mode)
    st_d = nc.dram_tensor("st8", [128, NJT, 2, 2, 128], f8, kind="ExternalInput")
    # moving X^T, i-rotated per core, iq-major [pk, iq, kh, dr, i]
    xt_d = nc.dram_tensor("xt8", [128, NIQ, 2, 2, IW], f8, kind="ExternalInput")
    rsqT_d = nc.dram_tensor("rsqT", [128, NJT], f32, kind="ExternalInput")
    # eyes[:, 0] = I*64, eyes[:, 1] = I*(-448)
    eyes_d = nc.dram_tensor("eyes", [128, 2, 128], f8, kind="ExternalInput")
    oA_d = nc.dram_tensor("oA", [128, B], f16, kind="ExternalOutput")

    with ExitStack() as ctx:
        tc = ctx.enter_context(tile.TileContext(nc))
        const = ctx.enter_context(tc.tile_pool(name="const", bufs=1))
        hpp = ctx.enter_context(tc.tile_pool(name="hp", bufs=3))
        accp = ctx.enter_context(tc.tile_pool(name="acc", bufs=2))
        psump = ctx.enter_context(tc.tile_pool(name="psum", bufs=4, space="PSUM"))

        def dummy_mm(lhs_ap, rhs_ap):
            pw = psump.tile([128, IW], f32, tag="q", name="pdum")
            nc.tensor.matmul(pw[: lhs_ap.shape[-1], : rhs_ap.shape[-1]],
                             lhs_ap, rhs_ap, start=True, stop=True)

        # --- input DMA launches, spread across engines so the descriptors
        # generate in parallel ---
        garb = const.tile([128, 512], f8, tag="garb")
        nc.vector.memset(garb, 0.0)
        eyes = const.tile([128, 2, 128], f8, tag="eyes")
        rsqT = const.tile([128, NJT], f32, tag="rsqT")
        xt = const.tile([128, NIQ, 2, 2, IW], f8, tag="xt")
        st = const.tile([128, NJT, 2, 2, 128], f8, tag="st")

        # need-ordered queues: the loop start only waits on xt[iq0,kh0] +
        # st[0:2] + eyes, so those get dedicated queues with nothing big ahead
        nc.sync.dma_start(out=xt[:, 0, 0], in_=xt_d[:, 0, 0])
        nc.sync.dma_start(out=xt[:, 0, 1], in_=xt_d[:, 0, 1])
        nc.sync.dma_start(out=xt[:, 1], in_=xt_d[:, 1])
        nc.sync.dma_start(out=xt[:, 2], in_=xt_d[:, 2])
        nc.sync.dma_start(out=xt[:, 3], in_=xt_d[:, 3])
        nc.scalar.dma_start(out=eyes, in_=eyes_d[:, :])
        nc.scalar.dma_start(out=rsqT, in_=rsqT_d[:, :])
        nc.scalar.dma_start(out=st[:, 0:2], in_=st_d[:, 0:2])
        nc.scalar.dma_start(out=st[:, 2:6], in_=st_d[:, 2:6])
        nc.scalar.dma_start(out=st[:, 6:NJT], in_=st_d[:, 6:NJT])

        # warmup burst on garbage data: ramps the PE HAM clock while the
        # input DMAs land (PE must stay busy ~3.4us to reach 2.4 GHz)
        for _ in range(7):
            pw = psump.tile([128, IW], f32, tag="q", name="pwarm")
            nc.tensor.matmul(pw[:, 0:512], garb[:, 0:128], garb[:, 0:512],
                             start=True, stop=True)

        # absorb DMA-completion waits on the consuming engines early
        dummy_mm(st[:, 0, 0, 0, 0:4], st[:, 0, 0, 0, 0:8])
        dummy_mm(xt[:, 0, 0, 0, 0:4], xt[:, 0, 0, 0, 0:8])
        dummy_mm(eyes[:, 0, 0:4], eyes[:, 0, 0:8])
        vabs = const.tile([128, 1], f32, tag="vabs")
        nc.vector.tensor_copy(vabs, rsqT[:, 0:1])
        sabs = const.tile([128, 1], f32, tag="sabs")
        nc.scalar.copy(sabs, rsqT[:, 0:1])

        for iq in range(NIQ):
            acc = accp.tile([128, IW], f16, tag="acc")
            for jt in (JT_TAIL_ORDER if iq == NIQ - 1 else range(NJT)):
                q = psump.tile([128, IW], f32, tag="q", name="q")
                # diagonal (self) block of this j-tile, if it lands in iq
                moff = jt * 128 - iq * IW
                has_mask = 0 <= moff < IW
                for kh in range(2):
                    for ib in range(2):
                        nc.tensor.matmul(
                            q[:, ib * 512 : (ib + 1) * 512],
                            st[:, jt, kh],
                            xt[:, iq, kh, :, ib * 512 : (ib + 1) * 512],
                            start=(kh == 0),
                            stop=(kh == 1 and not has_mask),
                            perf_mode=PM,
                            skip_group_check=has_mask,
                        )
                if has_mask:
                    nc.tensor.matmul(
                        q[:, moff : moff + 128],
                        eyes[:, 0],
                        eyes[:, 1],
                        start=False,
                        stop=True,
                        skip_group_check=True,
                    )
                if jt == 0:
                    nc.scalar.add(acc, q, rsqT[:, 0:1])
                elif jt in D_SET:
                    nc.vector.scalar_tensor_tensor(
                        out=acc, in0=q, scalar=rsqT[:, jt : jt + 1], in1=acc,
                        op0=add, op1=mx,
                    )
                else:
                    hp = hpp.tile([128, IW], f16, tag="hp")
                    nc.scalar.add(hp, q, rsqT[:, jt : jt + 1])
                    nc.vector.tensor_max(acc, acc, hp)
            nc.sync.dma_start(out=oA_d[:, iq * IW : (iq + 1) * IW], in_=acc)

    _split_multi_waits(nc)
    return nc


def _host_inputs(feature_ts, feature_image1, feature_image2):
    ts = np.ascontiguousarray(feature_ts, dtype=np.float32)
    im1 = np.ascontiguousarray(feature_image1, dtype=np.float32)
    im2 = np.ascontiguousarray(feature_image2, dtype=np.float32)

    R = np.concatenate([ts, im1, im2], 0)                      # [3B, D]
    rsq = (R.astype(np.float64) ** 2).sum(1)                   # [3B]
    x8 = ts.astype(F8)                                         # [B, D]
    r2_8 = (2.0 * R.astype(F8).astype(np.float32)).astype(F8)  # exact doubling

    eyes = np.zeros((128, 2, 128), dtype=F8)
    p = np.arange(128)
    eyes[p, 0, p] = F8(64.0)
    eyes[p, 1, p] = F8(-448.0)

    in_maps = []
    for c in range(M):
        jsl = slice(c * JC, (c + 1) * JC)
        # st5[pk, jt, kh, dr, j] = r2_8[c*JC + 128*jt + j, kh*256 + dr*128 + pk]
        st5 = r2_8[jsl].reshape(NJT, 128, 2, 2, 128).transpose(4, 0, 2, 3, 1)
        if PERF_MODE == "dri":
            # SwInterleave layout: per weight load the 256 bytes are
            # [A127, B127, A126, B126, ..., A0, B0] (A = dr0, B = dr1,
            # columns reversed)
            sti = np.empty((128, NJT, 2, 256), dtype=F8)
            sti[..., 0::2] = st5[..., 0, ::-1]
            sti[..., 1::2] = st5[..., 1, ::-1]
            sti = sti.reshape(128, NJT, 2, 2, 128)
        else:
            sti = np.ascontiguousarray(st5)
        # i-rotation: physical row ip holds logical row (ip + c*JC) % B, so
        # the self diagonal of j-tile jt sits at rows jt*128 + p on all cores
        iperm = (np.arange(B) + c * JC) % B
        xr = x8[iperm]                                         # [B, D] rotated
        # xt8[pk, iq, kh, dr, i] = xr[iq*IW + i, kh*256 + dr*128 + pk]
        xt8 = np.ascontiguousarray(
            xr.reshape(NIQ, IW, 2, 2, 128).transpose(4, 0, 2, 3, 1)
        )
        rsqT = np.ascontiguousarray(
            (-rsq[jsl].reshape(NJT, 128).T).astype(np.float32)
        )
        in_maps.append({"st8": sti, "xt8": xt8, "rsqT": rsqT, "eyes": eyes})
    return in_maps


def _combine(outs, feature_ts, feature_image1, feature_image2):
    ts = np.asarray(feature_ts, dtype=np.float64)
    im1 = np.asarray(feature_image1, dtype=np.float64)
    im2 = np.asarray(feature_image2, dtype=np.float64)
    l1 = np.sqrt(((ts - im1) ** 2).sum(1))
    l2 = np.sqrt(((ts - im2) ** 2).sum(1))
    d12 = np.sqrt(((im1 - im2 + 1e-6) ** 2).sum(1))
    xsq = (ts ** 2).sum(1)

    smax = np.full(B, -np.inf)
    for c, oA in enumerate(outs):
        s = np.asarray(oA, dtype=np.float64).max(axis=0)        # [B] physical i
        phys = (np.arange(B) + c * JC) % B
        np.maximum.at(smax, phys, s)
    negsq = xsq - smax
    neg = np.sqrt(np.maximum(negsq, 0.0))
    trip = np.maximum(l1 + l2 + d12 - neg + 0.1, 0.0) + np.maximum(l1, l2)
    return np.float32(trip.sum() / B)


def kernel(feature_ts, feature_image1, feature_image2, _trace=False):
    global _NC_CACHE, _NC_CACHE_MODE, LAST_RESULTS
    if _NC_CACHE is None or _NC_CACHE_MODE != PERF_MODE:
        _NC_CACHE = _build_nc(PERF_MODE)
        _NC_CACHE_MODE = PERF_MODE
    if _trace:
        _install_ntff_hook()
    in_maps = _host_inputs(feature_ts, feature_image1, feature_image2)
    res = run_bass_kernel_spmd(_NC_CACHE, in_maps, list(range(M)), trace=_trace)
    LAST_RESULTS = res
    return _combine(
        [res.results[c]["oA"] for c in range(M)],
        feature_ts, feature_image1, feature_image2,
    )


# revision 19
# speedup vs baseline: 1.2220x; 1.0052x over previous
"""DRCLoss kernel v4 for 8 Trainium2 NeuronCores (Bass/Tile, SPMD).

Math: loss = mean_i[ relu(l1_i + l2_i + d12_i - neg_i + 0.1) + max(l1_i, l2_i) ]
  where neg_i = min over non-self columns of cdist(ts, [ts; im1; im2])[i, :].

v4 strategy (column-sharded: each core owns 1536 of the 12288 columns j,
all 4096 rows i):
  - Device computes s[j, i] = 2*r_j.x_i - rsq[j] with output partitions = j.
    Only the masked column-max of s survives on device (negsq = xsq - max);
    l1/l2/d12/xsq are exact host math.
  - i is processed in 4 quarters of 1024; per (iq, jt) quad the PSUM block is
    [128, 1024] (2 banks), filled by 4 DR matmuls (2 kh x 2 ib of 512).
    Output per quarter streams out early, so the tail only exposes the last
    quarter's fold + a 256 KB DMA.
  - Single fp16 accumulator per quarter; fold paths balanced across engines:
      jt==0 : ScalarE activation copy psum+(-rsq) -> acc   (no max needed)
      jt in D_SET: DVE scalar_tensor_tensor acc = max(acc, psum + (-rsq))
      else  : ScalarE psum+(-rsq) -> fp16 tmp, DVE tensor_max fold
  - Self-column exclusion: per-core i-rotation puts the self diagonal of
    j-tile jt at rows jt*128+p; one tiny N=128 plain-fp8 matmul
    (eye*64 x eye*(-448)) adds -28672 there in PSUM.
  - Input DMA launches are spread across engines (sync/scalar/vector/gpsimd)
    and chunked so the first matmul only waits for st[jt0..5] + xt[iq0];
    a garbage warmup burst ramps the PE HAM clock during the DMA wait.
  - Host finishes: negsq = xsq - max over cores/partitions, sqrt/relu/mean
    in float64.
"""

import sys

if "/opt/trn_rl_repo" not in sys.path:
    sys.path.insert(0, "/opt/trn_rl_repo")

from contextlib import ExitStack

import ml_dtypes
import numpy as np

import concourse.bass as bass
import concourse.tile as tile
from concourse import mybir
from concourse.bass_utils import run_bass_kernel_spmd

BF16 = ml_dtypes.bfloat16
F8 = ml_dtypes.float8_e4m3

B = 4096          # rows
D = 512           # feature dim
M = 8             # cores
JC = 3 * B // M   # columns per core (1536)
NJT = JC // 128   # j-tiles per core (12)
NIQ = 4           # i quarters
IW = B // NIQ     # rows per quarter (1024)

# "dr" = hardware DoubleRow; "dri" = DoubleRowSwInterleave (host-interleaved
# weights so LDWEIGHTS streams contiguously)
PERF_MODE = "dr"

# j-tiles folded via the DVE scalar_tensor_tensor path (rest: ScalarE copy +
# DVE fp16 max). jt==0 is always the ScalarE direct-copy path. In the last
# i-quarter the final two folds are both single-op stt so the exposed tail
# after the last matmul is just one 1024-wide DVE op.
D_SET = (3, 8, 11)
D_SET_LAST = (3, 10, 11)

LAST_RESULTS = None

_NC_CACHE = None
_NC_CACHE_MODE = None


def _install_ntff_hook():
    """Provide antenv.axon_hooks (missing in this image) so trace=True can
    capture NTFF profiles through libaxon_pjrt.so."""
    try:
        import antenv.axon_hooks  # noqa: F401

        return
    except ImportError:
        pass
    try:
        import types

        import antenv
        from trn_agent_boot.trn_boot import _ntff_profile_via_ctypes

        mod = types.ModuleType("antenv.axon_hooks")
        mod._hook = None

        def set_axon_ntff_profile_hook(h):
            mod._hook = h

        def get_axon_ntff_profile_hook():
            return mod._hook

        mod.set_axon_ntff_profile_hook = set_axon_ntff_profile_hook
        mod.get_axon_ntff_profile_hook = get_axon_ntff_profile_hook
        sys.modules["antenv.axon_hooks"] = mod
        antenv.axon_hooks = mod
        hook = _ntff_profile_via_ctypes("/opt/axon/libaxon_pjrt.so")
        if hook is not None:
            mod._hook = hook
    except Exception:
        pass


def _split_multi_waits(nc):
    """This walrus build allows only ONE embedded sync wait per instruction.
    Hoist extra waits onto standalone EventSemaphore instructions inserted
    just before the owner (same engine, so program order is preserved)."""
    import bass_rust

    ctr = 0
    for blk in nc.m.functions[0].blocks:
        il = blk.instructions
        new = []
        for inst in il:
            si = getattr(inst, "sync_info", None)
            waits = list(si.on_wait) if si is not None else []
            if len(waits) > 1:
                for w in waits[:-1]:
                    ev = bass_rust.InstEventSemaphore(name=f"wsplit_{ctr}")
                    ctr += 1
                    ev.engine = inst.engine
                    ev.sync_info = bass_rust.SyncInfo(on_wait=[w], on_update=[])
                    new.append(ev)
                inst.sync_info = bass_rust.SyncInfo(
                    on_wait=[waits[-1]], on_update=list(si.on_update)
                )
            new.append(inst)
        il[:] = new


def _build_nc(perf_mode):
    nc = bass.Bass()
    f32 = mybir.dt.float32
    f16 = mybir.dt.float16
    f8 = mybir.dt.float8e4
    if perf_mode == "dri":
        PM = mybir.MatmulPerfMode.DoubleRowSwInterleave
    else:
        PM = mybir.MatmulPerfMode.DoubleRow
    add = mybir.AluOpType.add
    mx = mybir.AluOpType.max

    # stationary R-chunks [pk, jt, kh, dr, j] (dr x j byte order per perf mode)
    st_d = nc.dram_tensor("st8", [128, NJT, 2, 2, 128], f8, kind="ExternalInput")
    # moving X^T, i-rotated per core, iq-major [pk, iq, kh, dr, i]
    xt_d = nc.dram_tensor("xt8", [128, NIQ, 2, 2, IW], f8, kind="ExternalInput")
    rsqT_d = nc.dram_tensor("rsqT", [128, NJT], f32, kind="ExternalInput")
    # eyes[:, 0] = I*64, eyes[:, 1] = I*(-448)
    eyes_d = nc.dram_tensor("eyes", [128, 2, 128], f8, kind="ExternalInput")
    oA_d = nc.dram_tensor("oA", [128, B], f16, kind="ExternalOutput")

    with ExitStack() as ctx:
        tc = ctx.enter_context(tile.TileContext(nc))
        const = ctx.enter_context(tc.tile_pool(name="const", bufs=1))
        hpp = ctx.enter_context(tc.tile_pool(name="hp", bufs=3))
        accp = ctx.enter_context(tc.tile_pool(name="acc", bufs=2))
        psump = ctx.enter_context(tc.tile_pool(name="psum", bufs=4, space="PSUM"))

        def dummy_mm(lhs_ap, rhs_ap):
            pw = psump.tile([128, IW], f32, tag="q", name="pdum")
            nc.tensor.matmul(pw[: lhs_ap.shape[-1], : rhs_ap.shape[-1]],
                             lhs_ap, rhs_ap, start=True, stop=True)

        # --- input DMA launches, spread across engines so the descriptors
        # generate in parallel ---
        garb = const.tile([128, 512], f8, tag="garb")
        nc.vector.memset(garb, 0.0)
        eyes = const.tile([128, 2, 128], f8, tag="eyes")
        rsqT = const.tile([128, NJT], f32, tag="rsqT")

        # one tile per DMA chunk: tile-framework dependencies are whole-tile,
        # so a shared tile would make the first matmul wait for every chunk
        xq0k0 = const.tile([128, 2, IW], f8, tag="xq0k0")
        xq0k1 = const.tile([128, 2, IW], f8, tag="xq0k1")
        xq1 = const.tile([128, 2, 2, IW], f8, tag="xq1")
        xq2 = const.tile([128, 2, 2, IW], f8, tag="xq2")
        xq3 = const.tile([128, 2, 2, IW], f8, tag="xq3")
        stA = const.tile([128, 2, 2, 2, 128], f8, tag="stA")    # jt 0-1
        stB = const.tile([128, 4, 2, 2, 128], f8, tag="stB")    # jt 2-5
        stC = const.tile([128, 6, 2, 2, 128], f8, tag="stC")    # jt 6-11

        def st_ap(jt, kh):
            if jt < 2:
                return stA[:, jt, kh]
            if jt < 6:
                return stB[:, jt - 2, kh]
            return stC[:, jt - 6, kh]

        def xt_ap(iq, kh):
            if iq == 0:
                return (xq0k0 if kh == 0 else xq0k1)[:, :]
            return (xq1, xq2, xq3)[iq - 1][:, kh]

        # need-ordered queues: the loop start only waits on xt[iq0,kh0] +
        # st[0:2] + eyes, so those go first with nothing big ahead
        nc.sync.dma_start(out=xq0k0, in_=xt_d[:, 0, 0])
        nc.sync.dma_start(out=xq0k1, in_=xt_d[:, 0, 1])
        nc.sync.dma_start(out=xq1, in_=xt_d[:, 1])
        nc.sync.dma_start(out=xq2, in_=xt_d[:, 2])
        nc.sync.dma_start(out=xq3, in_=xt_d[:, 3])
        nc.scalar.dma_start(out=eyes, in_=eyes_d[:, :])
        nc.scalar.dma_start(out=rsqT, in_=rsqT_d[:, :])
        nc.scalar.dma_start(out=stA, in_=st_d[:, 0:2])
        nc.scalar.dma_start(out=stB, in_=st_d[:, 2:6])
        nc.scalar.dma_start(out=stC, in_=st_d[:, 6:NJT])

        # warmup burst on garbage data: ramps the PE HAM clock while the
        # input DMAs land (PE must stay busy ~3.4us to reach 2.4 GHz)
        for _ in range(7):
            pw = psump.tile([128, IW], f32, tag="q", name="pwarm")
            nc.tensor.matmul(pw[:, 0:512], garb[:, 0:128], garb[:, 0:512],
                             start=True, stop=True)

        # absorb DMA-completion waits on the consuming engines early
        dummy_mm(stA[:, 0, 0, 0, 0:4], stA[:, 0, 0, 0, 0:8])
        dummy_mm(xq0k0[:, 0, 0:4], xq0k0[:, 0, 0:8])
        dummy_mm(eyes[:, 0, 0:4], eyes[:, 0, 0:8])
        vabs = const.tile([128, 1], f32, tag="vabs")
        nc.vector.tensor_copy(vabs, rsqT[:, 0:1])
        sabs = const.tile([128, 1], f32, tag="sabs")
        nc.scalar.copy(sabs, rsqT[:, 0:1])

        for iq in range(NIQ):
            dset = D_SET_LAST if iq == NIQ - 1 else D_SET
            acc = accp.tile([128, IW], f16, tag="acc")
            for jt in range(NJT):
                q = psump.tile([128, IW], f32, tag="q", name="q")
                # diagonal (self) block of this j-tile, if it lands in iq
                moff = jt * 128 - iq * IW
                has_mask = 0 <= moff < IW
                for kh in range(2):
                    for ib in range(2):
                        nc.tensor.matmul(
                            q[:, ib * 512 : (ib + 1) * 512],
                            st_ap(jt, kh),
                            xt_ap(iq, kh)[:, :, ib * 512 : (ib + 1) * 512],
                            start=(kh == 0),
                            stop=(kh == 1 and not has_mask),
                            perf_mode=PM,
                            skip_group_check=has_mask,
                        )
                if has_mask:
                    nc.tensor.matmul(
                        q[:, moff : moff + 128],
                        eyes[:, 0],
                        eyes[:, 1],
                        start=False,
                        stop=True,
                        skip_group_check=True,
                    )
                if jt == 0:
                    nc.scalar.add(acc, q, rsqT[:, 0:1])
                elif jt in dset:
                    nc.vector.scalar_tensor_tensor(
                        out=acc, in0=q, scalar=rsqT[:, jt : jt + 1], in1=acc,
                        op0=add, op1=mx,
                    )
                else:
                    hp = hpp.tile([128, IW], f16, tag="hp")
                    nc.scalar.add(hp, q, rsqT[:, jt : jt + 1])
                    nc.vector.tensor_max(acc, acc, hp)
            nc.sync.dma_start(out=oA_d[:, iq * IW : (iq + 1) * IW], in_=acc)

    _split_multi_waits(nc)
    return nc


def _host_inputs(feature_ts, feature_image1, feature_image2):
    ts = np.ascontiguousarray(feature_ts, dtype=np.float32)
    im1 = np.ascontiguousarray(feature_image1, dtype=np.float32)
    im2 = np.ascontiguousarray(feature_image2, dtype=np.float32)

    R = np.concatenate([ts, im1, im2], 0)                      # [3B, D]
    rsq = (R.astype(np.float64) ** 2).sum(1)                   # [3B]
    x8 = ts.astype(F8)                                         # [B, D]
    r2_8 = (2.0 * R.astype(F8).astype(np.float32)).astype(F8)  # exact doubling

    eyes = np.zeros((128, 2, 128), dtype=F8)
    p = np.arange(128)
    eyes[p, 0, p] = F8(64.0)
    eyes[p, 1, p] = F8(-448.0)

    in_maps = []
    for c in range(M):
        jsl = slice(c * JC, (c + 1) * JC)
        # st5[pk, jt, kh, dr, j] = r2_8[c*JC + 128*jt + j, kh*256 + dr*128 + pk]
        st5 = r2_8[jsl].reshape(NJT, 128, 2, 2, 128).transpose(4, 0, 2, 3, 1)
        if PERF_MODE == "dri":
            # SwInterleave layout: per weight load the 256 bytes are
            # [A127, B127, A126, B126, ..., A0, B0] (A = dr0, B = dr1,
            # columns reversed)
            sti = np.empty((128, NJT, 2, 256), dtype=F8)
            sti[..., 0::2] = st5[..., 0, ::-1]
            sti[..., 1::2] = st5[..., 1, ::-1]
            sti = sti.reshape(128, NJT, 2, 2, 128)
        else:
            sti = np.ascontiguousarray(st5)
        # i-rotation: physical row ip holds logical row (ip + c*JC) % B, so
        # the self diagonal of j-tile jt sits at rows jt*128 + p on all cores
        iperm = (np.arange(B) + c * JC) % B
        xr = x8[iperm]                                         # [B, D] rotated
        # xt8[pk, iq, kh, dr, i] = xr[iq*IW + i, kh*256 + dr*128 + pk]
        xt8 = np.ascontiguousarray(
            xr.reshape(NIQ, IW, 2, 2, 128).transpose(4, 0, 2, 3, 1)
        )
        rsqT = np.ascontiguousarray(
            (-rsq[jsl].reshape(NJT, 128).T).astype(np.float32)
        )
        in_maps.append({"st8": sti, "xt8": xt8, "rsqT": rsqT, "eyes": eyes})
    return in_maps


def _combine(outs, feature_ts, feature_image1, feature_image2):
    ts = np.asarray(feature_ts, dtype=np.float64)
    im1 = np.asarray(feature_image1, dtype=np.float64)
    im2 = np.asarray(feature_image2, dtype=np.float64)
    l1 = np.sqrt(((ts - im1) ** 2).sum(1))
    l2 = np.sqrt(((ts - im2) ** 2).sum(1))
    d12 = np.sqrt(((im1 - im2 + 1e-6) ** 2).sum(1))
    xsq = (ts ** 2).sum(1)

    smax = np.full(B, -np.inf)
    for c, oA in enumerate(outs):
        s = np.asarray(oA, dtype=np.float64).max(axis=0)        # [B] physical i
        phys = (np.arange(B) + c * JC) % B
        np.maximum.at(smax, phys, s)
    negsq = xsq - smax
    neg = np.sqrt(np.maximum(negsq, 0.0))
    trip = np.maximum(l1 + l2 + d12 - neg + 0.1, 0.0) + np.maximum(l1, l2)
    return np.float32(trip.sum() / B)


def kernel(feature_ts, feature_image1, feature_image2, _trace=False):
    global _NC_CACHE, _NC_CACHE_MODE, LAST_RESULTS
    if _NC_CACHE is None or _NC_CACHE_MODE != PERF_MODE:
        _NC_CACHE = _build_nc(PERF_MODE)
        _NC_CACHE_MODE = PERF_MODE
    if _trace:
        _install_ntff_hook()
    in_maps = _host_inputs(feature_ts, feature_image1, feature_image2)
    res = run_bass_kernel_spmd(_NC_CACHE, in_maps, list(range(M)), trace=_trace)
    LAST_RESULTS = res
    return _combine(
        [res.results[c]["oA"] for c in range(M)],
        feature_ts, feature_image1, feature_image2,
    )


# revision 25
# speedup vs baseline: 1.2750x; 1.0433x over previous
"""DRCLoss kernel v4 for 8 Trainium2 NeuronCores (Bass/Tile, SPMD).

Math: loss = mean_i[ relu(l1_i + l2_i + d12_i - neg_i + 0.1) + max(l1_i, l2_i) ]
  where neg_i = min over non-self columns of cdist(ts, [ts; im1; im2])[i, :].

v4 strategy (column-sharded: each core owns 1536 of the 12288 columns j,
all 4096 rows i):
  - Device computes s[j, i] = 2*r_j.x_i - rsq[j] with output partitions = j.
    Only the masked column-max of s survives on device (negsq = xsq - max);
    l1/l2/d12/xsq are exact host math.
  - i is processed in 4 quarters of 1024; per (iq, jt) quad the PSUM block is
    [128, 1024] (2 banks), filled by 4 DR matmuls (2 kh x 2 ib of 512).
    Output per quarter streams out early, so the tail only exposes the last
    quarter's fold + a 256 KB DMA.
  - Single fp16 accumulator per quarter; fold paths balanced across engines:
      jt==0 : ScalarE activation copy psum+(-rsq) -> acc   (no max needed)
      jt in D_SET: DVE scalar_tensor_tensor acc = max(acc, psum + (-rsq))
      else  : ScalarE psum+(-rsq) -> fp16 tmp, DVE tensor_max fold
  - Self-column exclusion: per-core i-rotation puts the self diagonal of
    j-tile jt at rows jt*128+p; one tiny N=128 plain-fp8 matmul
    (eye*64 x eye*(-448)) adds -28672 there in PSUM.
  - Input DMA launches are spread across engines (sync/scalar/vector/gpsimd)
    and chunked so the first matmul only waits for st[jt0..5] + xt[iq0];
    a garbage warmup burst ramps the PE HAM clock during the DMA wait.
  - Host finishes: negsq = xsq - max over cores/partitions, sqrt/relu/mean
    in float64.
"""

import sys

if "/opt/trn_rl_repo" not in sys.path:
    sys.path.insert(0, "/opt/trn_rl_repo")

from contextlib import ExitStack

import ml_dtypes
import numpy as np

import concourse.bass as bass
import concourse.tile as tile
from concourse import mybir
from concourse.bass_utils import run_bass_kernel_spmd

BF16 = ml_dtypes.bfloat16
F8 = ml_dtypes.float8_e4m3

B = 4096          # rows
D = 512           # feature dim
M = 8             # cores
JC = 3 * B // M   # columns per core (1536)
NJT = JC // 128   # j-tiles per core (12)
NIQ = 4           # i quarters
IW = B // NIQ     # rows per quarter (1024)

# "dr" = hardware DoubleRow; "dri" = DoubleRowSwInterleave (host-interleaved
# weights so LDWEIGHTS streams contiguously)
PERF_MODE = "dr"

# Folds go into TWO accumulators (even jt -> accE, odd jt -> accO) so the
# per-acc serial fold chains have 2-quad spacing and never trail the matmuls.
# jt 0/1 initialize their chain via ScalarE copy; D_SET j-tiles fold via a
# single DVE scalar_tensor_tensor (includes 10/11 so the tail exposes only
# one stt); the rest use ScalarE copy + DVE fp16 max.
D_SET = (4, 8, 10, 11)

LAST_RESULTS = None

_NC_CACHE = None
_NC_CACHE_MODE = None


def _install_ntff_hook():
    """Provide antenv.axon_hooks (missing in this image) so trace=True can
    capture NTFF profiles through libaxon_pjrt.so."""
    try:
        import antenv.axon_hooks  # noqa: F401

        return
    except ImportError:
        pass
    try:
        import types

        import antenv
        from trn_agent_boot.trn_boot import _ntff_profile_via_ctypes

        mod = types.ModuleType("antenv.axon_hooks")
        mod._hook = None

        def set_axon_ntff_profile_hook(h):
            mod._hook = h

        def get_axon_ntff_profile_hook():
            return mod._hook

        mod.set_axon_ntff_profile_hook = set_axon_ntff_profile_hook
        mod.get_axon_ntff_profile_hook = get_axon_ntff_profile_hook
        sys.modules["antenv.axon_hooks"] = mod
        antenv.axon_hooks = mod
        hook = _ntff_profile_via_ctypes("/opt/axon/libaxon_pjrt.so")
        if hook is not None:
            mod._hook = hook
    except Exception:
        pass


def _split_multi_waits(nc):
    """This walrus build allows only ONE embedded sync wait per instruction.
    Hoist extra waits onto standalone EventSemaphore instructions inserted
    just before the owner (same engine, so program order is preserved)."""
    import bass_rust

    ctr = 0
    for blk in nc.m.functions[0].blocks:
        il = blk.instructions
        new = []
        for inst in il:
            si = getattr(inst, "sync_info", None)
            waits = list(si.on_wait) if si is not None else []
            if len(waits) > 1:
                for w in waits[:-1]:
                    ev = bass_rust.InstEventSemaphore(name=f"wsplit_{ctr}")
                    ctr += 1
                    ev.engine = inst.engine
                    ev.sync_info = bass_rust.SyncInfo(on_wait=[w], on_update=[])
                    new.append(ev)
                inst.sync_info = bass_rust.SyncInfo(
                    on_wait=[waits[-1]], on_update=list(si.on_update)
                )
            new.append(inst)
        il[:] = new


def _build_nc(perf_mode):
    nc = bass.Bass()
    f32 = mybir.dt.float32
    f16 = mybir.dt.float16
    f8 = mybir.dt.float8e4
    if perf_mode == "dri":
        PM = mybir.MatmulPerfMode.DoubleRowSwInterleave
    else:
        PM = mybir.MatmulPerfMode.DoubleRow
    add = mybir.AluOpType.add
    mx = mybir.AluOpType.max

    # stationary R-chunks [pk, jt, kh, dr, j] (dr x j byte order per perf mode)
    st_d = nc.dram_tensor("st8", [128, NJT, 2, 2, 128], f8, kind="ExternalInput")
    # moving X^T, i-rotated per core, iq-major [pk, iq, kh, dr, i]
    xt_d = nc.dram_tensor("xt8", [128, NIQ, 2, 2, IW], f8, kind="ExternalInput")
    rsqT_d = nc.dram_tensor("rsqT", [128, NJT], f32, kind="ExternalInput")
    # eyes[:, 0] = I*64, eyes[:, 1] = I*(-448)
    eyes_d = nc.dram_tensor("eyes", [128, 2, 128], f8, kind="ExternalInput")
    oE_d = nc.dram_tensor("oE", [128, B], f16, kind="ExternalOutput")
    oO_d = nc.dram_tensor("oO", [128, B], f16, kind="ExternalOutput")

    with ExitStack() as ctx:
        tc = ctx.enter_context(tile.TileContext(nc))
        const = ctx.enter_context(tc.tile_pool(name="const", bufs=1))
        hpp = ctx.enter_context(tc.tile_pool(name="hp", bufs=3))
        accp = ctx.enter_context(tc.tile_pool(name="acc", bufs=2))
        psump = ctx.enter_context(tc.tile_pool(name="psum", bufs=4, space="PSUM"))

        def dummy_mm(lhs_ap, rhs_ap):
            pw = psump.tile([128, IW], f32, tag="q", name="pdum")
            nc.tensor.matmul(pw[: lhs_ap.shape[-1], : rhs_ap.shape[-1]],
                             lhs_ap, rhs_ap, start=True, stop=True)

        # --- input DMA launches, spread across engines so the descriptors
        # generate in parallel ---
        garb = const.tile([128, 512], f8, tag="garb")
        nc.vector.memset(garb, 0.0)
        eyes = const.tile([128, 2, 128], f8, tag="eyes")
        rsqT = const.tile([128, NJT], f32, tag="rsqT")

        # one tile per DMA chunk: tile-framework dependencies are whole-tile,
        # so a shared tile would make the first matmul wait for every chunk
        xq0k0 = const.tile([128, 2, IW], f8, tag="xq0k0")
        xq0k1 = const.tile([128, 2, IW], f8, tag="xq0k1")
        xq1 = const.tile([128, 2, 2, IW], f8, tag="xq1")
        xq2 = const.tile([128, 2, 2, IW], f8, tag="xq2")
        xq3 = const.tile([128, 2, 2, IW], f8, tag="xq3")
        stA = const.tile([128, 2, 2, 2, 128], f8, tag="stA")    # jt 0-1
        stB = const.tile([128, 4, 2, 2, 128], f8, tag="stB")    # jt 2-5
        stC = const.tile([128, 6, 2, 2, 128], f8, tag="stC")    # jt 6-11

        def st_ap(jt, kh):
            if jt < 2:
                return stA[:, jt, kh]
            if jt < 6:
                return stB[:, jt - 2, kh]
            return stC[:, jt - 6, kh]

        def xt_ap(iq, kh):
            if iq == 0:
                return (xq0k0 if kh == 0 else xq0k1)[:, :]
            return (xq1, xq2, xq3)[iq - 1][:, kh]

        # need-ordered queues: the loop start only waits on xt[iq0,kh0] +
        # st[0:2] + eyes, so those go first with nothing big ahead
        nc.sync.dma_start(out=xq0k0, in_=xt_d[:, 0, 0])
        nc.sync.dma_start(out=xq0k1, in_=xt_d[:, 0, 1])
        nc.sync.dma_start(out=xq1, in_=xt_d[:, 1])
        nc.sync.dma_start(out=xq2, in_=xt_d[:, 2])
        nc.sync.dma_start(out=xq3, in_=xt_d[:, 3])
        nc.scalar.dma_start(out=stA, in_=st_d[:, 0:2])
        nc.scalar.dma_start(out=eyes, in_=eyes_d[:, :])
        nc.scalar.dma_start(out=rsqT, in_=rsqT_d[:, :])
        nc.scalar.dma_start(out=stB, in_=st_d[:, 2:6])
        nc.scalar.dma_start(out=stC, in_=st_d[:, 6:NJT])

        # warmup burst on garbage data: ramps the PE HAM clock while the
        # input DMAs land (PE must stay busy ~3.4us to reach 2.4 GHz)
        for _ in range(7):
            pw = psump.tile([128, IW], f32, tag="q", name="pwarm")
            nc.tensor.matmul(pw[:, 0:512], garb[:, 0:128], garb[:, 0:512],
                             start=True, stop=True)

        # absorb DMA-completion waits on the consuming engines early
        dummy_mm(stA[:, 0, 0, 0, 0:4], stA[:, 0, 0, 0, 0:8])
        dummy_mm(xq0k0[:, 0, 0:4], xq0k0[:, 0, 0:8])
        dummy_mm(eyes[:, 0, 0:4], eyes[:, 0, 0:8])
        vabs = const.tile([128, 1], f32, tag="vabs")
        nc.vector.tensor_copy(vabs, rsqT[:, 0:1])
        sabs = const.tile([128, 1], f32, tag="sabs")
        nc.scalar.copy(sabs, rsqT[:, 0:1])

        for iq in range(NIQ):
            accE = accp.tile([128, IW], f16, tag="accE")
            accO = accp.tile([128, IW], f16, tag="accO")
            for jt in range(NJT):
                acc = accO if jt % 2 else accE
                q = psump.tile([128, IW], f32, tag="q", name="q")
                # diagonal (self) block of this j-tile, if it lands in iq
                moff = jt * 128 - iq * IW
                has_mask = 0 <= moff < IW
                for kh in range(2):
                    for ib in range(2):
                        nc.tensor.matmul(
                            q[:, ib * 512 : (ib + 1) * 512],
                            st_ap(jt, kh),
                            xt_ap(iq, kh)[:, :, ib * 512 : (ib + 1) * 512],
                            start=(kh == 0),
                            stop=(kh == 1 and not has_mask),
                            perf_mode=PM,
                            skip_group_check=has_mask,
                        )
                if has_mask:
                    nc.tensor.matmul(
                        q[:, moff : moff + 128],
                        eyes[:, 0],
                        eyes[:, 1],
                        start=False,
                        stop=True,
                        skip_group_check=True,
                    )
                if jt < 2:
                    nc.scalar.add(acc, q, rsqT[:, jt : jt + 1])
                elif jt in D_SET:
                    nc.vector.scalar_tensor_tensor(
                        out=acc, in0=q, scalar=rsqT[:, jt : jt + 1], in1=acc,
                        op0=add, op1=mx,
                    )
                else:
                    hp = hpp.tile([128, IW], f16, tag="hp")
                    nc.scalar.add(hp, q, rsqT[:, jt : jt + 1])
                    nc.vector.tensor_max(acc, acc, hp)
            nc.sync.dma_start(out=oE_d[:, iq * IW : (iq + 1) * IW], in_=accE)
            nc.sync.dma_start(out=oO_d[:, iq * IW : (iq + 1) * IW], in_=accO)

    _split_multi_waits(nc)
    return nc


def _host_inputs(feature_ts, feature_image1, feature_image2):
    ts = np.ascontiguousarray(feature_ts, dtype=np.float32)
    im1 = np.ascontiguousarray(feature_image1, dtype=np.float32)
    im2 = np.ascontiguousarray(feature_image2, dtype=np.float32)

    R = np.concatenate([ts, im1, im2], 0)                      # [3B, D]
    rsq = (R.astype(np.float64) ** 2).sum(1)                   # [3B]
    x8 = ts.astype(F8)                                         # [B, D]
    r2_8 = (2.0 * R.astype(F8).astype(np.float32)).astype(F8)  # exact doubling

    eyes = np.zeros((128, 2, 128), dtype=F8)
    p = np.arange(128)
    eyes[p, 0, p] = F8(64.0)
    eyes[p, 1, p] = F8(-448.0)

    in_maps = []
    for c in range(M):
        jsl = slice(c * JC, (c + 1) * JC)
        # st5[pk, jt, kh, dr, j] = r2_8[c*JC + 128*jt + j, kh*256 + dr*128 + pk]
        st5 = r2_8[jsl].reshape(NJT, 128, 2, 2, 128).transpose(4, 0, 2, 3, 1)
        if PERF_MODE == "dri":
            # SwInterleave layout: per weight load the 256 bytes are
            # [A127, B127, A126, B126, ..., A0, B0] (A = dr0, B = dr1,
            # columns reversed)
            sti = np.empty((128, NJT, 2, 256), dtype=F8)
            sti[..., 0::2] = st5[..., 0, ::-1]
            sti[..., 1::2] = st5[..., 1, ::-1]
            sti = sti.reshape(128, NJT, 2, 2, 128)
        else:
            sti = np.ascontiguousarray(st5)
        # i-rotation: physical row ip holds logical row (ip + c*JC) % B, so
        # the self diagonal of j-tile jt sits at rows jt*128 + p on all cores
        iperm = (np.arange(B) + c * JC) % B
        xr = x8[iperm]                                         # [B, D] rotated
        # xt8[pk, iq, kh, dr, i] = xr[iq*IW + i, kh*256 + dr*128 + pk]
        xt8 = np.ascontiguousarray(
            xr.reshape(NIQ, IW, 2, 2, 128).transpose(4, 0, 2, 3, 1)
        )
        rsqT = np.ascontiguousarray(
            (-rsq[jsl].reshape(NJT, 128).T).astype(np.float32)
        )
        in_maps.append({"st8": sti, "xt8": xt8, "rsqT": rsqT, "eyes": eyes})
    return in_maps


def _combine(outs, feature_ts, feature_image1, feature_image2):
    ts = np.asarray(feature_ts, dtype=np.float64)
    im1 = np.asarray(feature_image1, dtype=np.float64)
    im2 = np.asarray(feature_image2, dtype=np.float64)
    l1 = np.sqrt(((ts - im1) ** 2).sum(1))
    l2 = np.sqrt(((ts - im2) ** 2).sum(1))
    d12 = np.sqrt(((im1 - im2 + 1e-6) ** 2).sum(1))
    xsq = (ts ** 2).sum(1)

    smax = np.full(B, -np.inf)
    for c, (oE, oO) in enumerate(outs):
        s = np.maximum(
            np.asarray(oE, dtype=np.float64).max(axis=0),
            np.asarray(oO, dtype=np.float64).max(axis=0),
        )                                                       # [B] physical i
        phys = (np.arange(B) + c * JC) % B
        np.maximum.at(smax, phys, s)
    negsq = xsq - smax
    neg = np.sqrt(np.maximum(negsq, 0.0))
    trip = np.maximum(l1 + l2 + d12 - neg + 0.1, 0.0) + np.maximum(l1, l2)
    return np.float32(trip.sum() / B)


def kernel(feature_ts, feature_image1, feature_image2, _trace=False):
    global _NC_CACHE, _NC_CACHE_MODE, LAST_RESULTS
    if _NC_CACHE is None or _NC_CACHE_MODE != PERF_MODE:
        _NC_CACHE = _build_nc(PERF_MODE)
        _NC_CACHE_MODE = PERF_MODE
    if _trace:
        _install_ntff_hook()
    in_maps = _host_inputs(feature_ts, feature_image1, feature_image2)
    res = run_bass_kernel_spmd(_NC_CACHE, in_maps, list(range(M)), trace=_trace)
    LAST_RESULTS = res
    return _combine(
        [(res.results[c]["oE"], res.results[c]["oO"]) for c in range(M)],
        feature_ts, feature_image1, feature_image2,
    )


# revision 28
# speedup vs baseline: 1.3286x; 1.0421x over previous
"""DRCLoss kernel v4 for 8 Trainium2 NeuronCores (Bass/Tile, SPMD).

Math: loss = mean_i[ relu(l1_i + l2_i + d12_i - neg_i + 0.1) + max(l1_i, l2_i) ]
  where neg_i = min over non-self columns of cdist(ts, [ts; im1; im2])[i, :].

v4 strategy (column-sharded: each core owns 1536 of the 12288 columns j,
all 4096 rows i):
  - Device computes s[j, i] = 2*r_j.x_i - rsq[j] with output partitions = j.
    Only the masked column-max of s survives on device (negsq = xsq - max);
    l1/l2/d12/xsq are exact host math.
  - i is processed in 4 quarters of 1024; per (iq, jt) quad the PSUM block is
    [128, 1024] (2 banks), filled by 4 DR matmuls (2 kh x 2 ib of 512).
    Output per quarter streams out early, so the tail only exposes the last
    quarter's fold + a 256 KB DMA.
  - Single fp16 accumulator per quarter; fold paths balanced across engines:
      jt==0 : ScalarE activation copy psum+(-rsq) -> acc   (no max needed)
      jt in D_SET: DVE scalar_tensor_tensor acc = max(acc, psum + (-rsq))
      else  : ScalarE psum+(-rsq) -> fp16 tmp, DVE tensor_max fold
  - Self-column exclusion: per-core i-rotation puts the self diagonal of
    j-tile jt at rows jt*128+p; one tiny N=128 plain-fp8 matmul
    (eye*64 x eye*(-448)) adds -28672 there in PSUM.
  - Input DMA launches are spread across engines (sync/scalar/vector/gpsimd)
    and chunked so the first matmul only waits for st[jt0..5] + xt[iq0];
    a garbage warmup burst ramps the PE HAM clock during the DMA wait.
  - Host finishes: negsq = xsq - max over cores/partitions, sqrt/relu/mean
    in float64.
"""

import sys

if "/opt/trn_rl_repo" not in sys.path:
    sys.path.insert(0, "/opt/trn_rl_repo")

from contextlib import ExitStack

import ml_dtypes
import numpy as np

import concourse.bass as bass
import concourse.tile as tile
from concourse import mybir
from concourse.bass_utils import run_bass_kernel_spmd

BF16 = ml_dtypes.bfloat16
F8 = ml_dtypes.float8_e4m3

B = 4096          # rows
D = 512           # feature dim
M = 8             # cores
JC = 3 * B // M   # columns per core (1536)
NJT = JC // 128   # j-tiles per core (12)
NIQ = 4           # i quarters
IW = B // NIQ     # rows per quarter (1024)

# "dr" = hardware DoubleRow; "dri" = DoubleRowSwInterleave (host-interleaved
# weights so LDWEIGHTS streams contiguously)
PERF_MODE = "dr"

# Folds go into TWO accumulators (even jt -> accE, odd jt -> accO) so the
# per-acc serial fold chains have 2-quad spacing and never trail the matmuls.
# jt 0/1 initialize their chain via ScalarE copy; D_SET j-tiles fold via a
# single DVE scalar_tensor_tensor (early/mid only: the slow 1.28us stt ops
# must not stack at the iq end); the rest use ScalarE copy + DVE fp16 max,
# whose trailing ops pipeline behind the closing matmuls.
D_SET = (2, 4, 6, 8)

LAST_RESULTS = None

_NC_CACHE = None
_NC_CACHE_MODE = None


def _install_ntff_hook():
    """Provide antenv.axon_hooks (missing in this image) so trace=True can
    capture NTFF profiles through libaxon_pjrt.so."""
    try:
        import antenv.axon_hooks  # noqa: F401

        return
    except ImportError:
        pass
    try:
        import types

        import antenv
        from trn_agent_boot.trn_boot import _ntff_profile_via_ctypes

        mod = types.ModuleType("antenv.axon_hooks")
        mod._hook = None

        def set_axon_ntff_profile_hook(h):
            mod._hook = h

        def get_axon_ntff_profile_hook():
            return mod._hook

        mod.set_axon_ntff_profile_hook = set_axon_ntff_profile_hook
        mod.get_axon_ntff_profile_hook = get_axon_ntff_profile_hook
        sys.modules["antenv.axon_hooks"] = mod
        antenv.axon_hooks = mod
        hook = _ntff_profile_via_ctypes("/opt/axon/libaxon_pjrt.so")
        if hook is not None:
            mod._hook = hook
    except Exception:
        pass


def _split_multi_waits(nc):
    """This walrus build allows only ONE embedded sync wait per instruction.
    Hoist extra waits onto standalone EventSemaphore instructions inserted
    just before the owner (same engine, so program order is preserved)."""
    import bass_rust

    ctr = 0
    for blk in nc.m.functions[0].blocks:
        il = blk.instructions
        new = []
        for inst in il:
            si = getattr(inst, "sync_info", None)
            waits = list(si.on_wait) if si is not None else []
            if len(waits) > 1:
                for w in waits[:-1]:
                    ev = bass_rust.InstEventSemaphore(name=f"wsplit_{ctr}")
                    ctr += 1
                    ev.engine = inst.engine
                    ev.sync_info = bass_rust.SyncInfo(on_wait=[w], on_update=[])
                    new.append(ev)
                inst.sync_info = bass_rust.SyncInfo(
                    on_wait=[waits[-1]], on_update=list(si.on_update)
                )
            new.append(inst)
        il[:] = new


def _build_nc(perf_mode):
    nc = bass.Bass()
    f32 = mybir.dt.float32
    f16 = mybir.dt.float16
    f8 = mybir.dt.float8e4
    if perf_mode == "dri":
        PM = mybir.MatmulPerfMode.DoubleRowSwInterleave
    else:
        PM = mybir.MatmulPerfMode.DoubleRow
    add = mybir.AluOpType.add
    mx = mybir.AluOpType.max

    # stationary R-chunks [pk, jt, kh, dr, j] (dr x j byte order per perf mode)
    st_d = nc.dram_tensor("st8", [128, NJT, 2, 2, 128], f8, kind="ExternalInput")
    # moving X^T, i-rotated per core, iq-major [pk, iq, kh, dr, i]
    xt_d = nc.dram_tensor("xt8", [128, NIQ, 2, 2, IW], f8, kind="ExternalInput")
    rsqT_d = nc.dram_tensor("rsqT", [128, NJT], f32, kind="ExternalInput")
    # eyes[:, 0] = I*64, eyes[:, 1] = I*(-448)
    eyes_d = nc.dram_tensor("eyes", [128, 2, 128], f8, kind="ExternalInput")
    oE_d = nc.dram_tensor("oE", [128, B], f16, kind="ExternalOutput")
    oO_d = nc.dram_tensor("oO", [128, B], f16, kind="ExternalOutput")

    with ExitStack() as ctx:
        tc = ctx.enter_context(tile.TileContext(nc))
        const = ctx.enter_context(tc.tile_pool(name="const", bufs=1))
        hpp = ctx.enter_context(tc.tile_pool(name="hp", bufs=3))
        accp = ctx.enter_context(tc.tile_pool(name="acc", bufs=2))
        psump = ctx.enter_context(tc.tile_pool(name="psum", bufs=4, space="PSUM"))

        def dummy_mm(lhs_ap, rhs_ap):
            pw = psump.tile([128, IW], f32, tag="q", name="pdum")
            nc.tensor.matmul(pw[: lhs_ap.shape[-1], : rhs_ap.shape[-1]],
                             lhs_ap, rhs_ap, start=True, stop=True)

        # --- input DMA launches, spread across engines so the descriptors
        # generate in parallel ---
        garb = const.tile([128, 512], f8, tag="garb")
        nc.vector.memset(garb, 0.0)
        eyes = const.tile([128, 2, 128], f8, tag="eyes")
        rsqT = const.tile([128, NJT], f32, tag="rsqT")

        # one tile per DMA chunk: tile-framework dependencies are whole-tile,
        # so a shared tile would make the first matmul wait for every chunk
        xq0k0 = const.tile([128, 2, IW], f8, tag="xq0k0")
        xq0k1 = const.tile([128, 2, IW], f8, tag="xq0k1")
        xq1 = const.tile([128, 2, 2, IW], f8, tag="xq1")
        xq2 = const.tile([128, 2, 2, IW], f8, tag="xq2")
        xq3 = const.tile([128, 2, 2, IW], f8, tag="xq3")
        stA = const.tile([128, 3, 2, 2, 128], f8, tag="stA")    # jt 0-2
        stB = const.tile([128, 4, 2, 2, 128], f8, tag="stB")    # jt 3-6
        stC = const.tile([128, 5, 2, 2, 128], f8, tag="stC")    # jt 7-11

        def st_ap(jt, kh):
            if jt < 3:
                return stA[:, jt, kh]
            if jt < 7:
                return stB[:, jt - 3, kh]
            return stC[:, jt - 7, kh]

        def xt_ap(iq, kh):
            if iq == 0:
                return (xq0k0 if kh == 0 else xq0k1)[:, :]
            return (xq1, xq2, xq3)[iq - 1][:, kh]

        # need-ordered queues: the loop start only waits on xt[iq0,kh0] +
        # st[0:2] + eyes, so those go first with nothing big ahead
        nc.sync.dma_start(out=xq0k0, in_=xt_d[:, 0, 0])
        nc.sync.dma_start(out=xq0k1, in_=xt_d[:, 0, 1])
        nc.sync.dma_start(out=xq1, in_=xt_d[:, 1])
        nc.sync.dma_start(out=xq2, in_=xt_d[:, 2])
        nc.sync.dma_start(out=xq3, in_=xt_d[:, 3])
        nc.scalar.dma_start(out=stA, in_=st_d[:, 0:3])
        nc.scalar.dma_start(out=eyes, in_=eyes_d[:, :])
        nc.scalar.dma_start(out=rsqT, in_=rsqT_d[:, :])
        nc.scalar.dma_start(out=stB, in_=st_d[:, 3:7])
        nc.scalar.dma_start(out=stC, in_=st_d[:, 7:NJT])

        # warmup burst on garbage data: ramps the PE HAM clock while the
        # input DMAs land (PE must stay busy ~3.4us to reach 2.4 GHz)
        for _ in range(7):
            pw = psump.tile([128, IW], f32, tag="q", name="pwarm")
            nc.tensor.matmul(pw[:, 0:512], garb[:, 0:128], garb[:, 0:512],
                             start=True, stop=True)

        # absorb DMA-completion waits on the consuming engines early
        dummy_mm(stA[:, 0, 0, 0, 0:4], stA[:, 0, 0, 0, 0:8])
        dummy_mm(xq0k0[:, 0, 0:4], xq0k0[:, 0, 0:8])
        dummy_mm(eyes[:, 0, 0:4], eyes[:, 0, 0:8])
        vabs = const.tile([128, 1], f32, tag="vabs")
        nc.vector.tensor_copy(vabs, rsqT[:, 0:1])
        sabs = const.tile([128, 1], f32, tag="sabs")
        nc.scalar.copy(sabs, rsqT[:, 0:1])

        for iq in range(NIQ):
            accE = accp.tile([128, IW], f16, tag="accE")
            accO = accp.tile([128, IW], f16, tag="accO")
            for jt in range(NJT):
                acc = accO if jt % 2 else accE
                q = psump.tile([128, IW], f32, tag="q", name="q")
                # diagonal (self) block of this j-tile, if it lands in iq
                moff = jt * 128 - iq * IW
                has_mask = 0 <= moff < IW
                for kh in range(2):
                    for ib in range(2):
                        nc.tensor.matmul(
                            q[:, ib * 512 : (ib + 1) * 512],
                            st_ap(jt, kh),
                            xt_ap(iq, kh)[:, :, ib * 512 : (ib + 1) * 512],
                            start=(kh == 0),
                            stop=(kh == 1 and not has_mask),
                            perf_mode=PM,
                            skip_group_check=has_mask,
                        )
                if has_mask:
                    nc.tensor.matmul(
                        q[:, moff : moff + 128],
                        eyes[:, 0],
                        eyes[:, 1],
                        start=False,
                        stop=True,
                        skip_group_check=True,
                    )
                if jt < 2:
                    nc.scalar.add(acc, q, rsqT[:, jt : jt + 1])
                elif jt in D_SET:
                    nc.vector.scalar_tensor_tensor(
                        out=acc, in0=q, scalar=rsqT[:, jt : jt + 1], in1=acc,
                        op0=add, op1=mx,
                    )
                else:
                    hp = hpp.tile([128, IW], f16, tag="hp")
                    nc.scalar.add(hp, q, rsqT[:, jt : jt + 1])
                    nc.vector.tensor_max(acc, acc, hp)
            nc.sync.dma_start(out=oE_d[:, iq * IW : (iq + 1) * IW], in_=accE)
            nc.sync.dma_start(out=oO_d[:, iq * IW : (iq + 1) * IW], in_=accO)

    _split_multi_waits(nc)
    return nc


def _host_inputs(feature_ts, feature_image1, feature_image2):
    ts = np.ascontiguousarray(feature_ts, dtype=np.float32)
    im1 = np.ascontiguousarray(feature_image1, dtype=np.float32)
    im2 = np.ascontiguousarray(feature_image2, dtype=np.float32)

    R = np.concatenate([ts, im1, im2], 0)                      # [3B, D]
    rsq = (R.astype(np.float64) ** 2).sum(1)                   # [3B]
    x8 = ts.astype(F8)                                         # [B, D]
    r2_8 = (2.0 * R.astype(F8).astype(np.float32)).astype(F8)  # exact doubling

    eyes = np.zeros((128, 2, 128), dtype=F8)
    p = np.arange(128)
    eyes[p, 0, p] = F8(64.0)
    eyes[p, 1, p] = F8(-448.0)

    in_maps = []
    for c in range(M):
        jsl = slice(c * JC, (c + 1) * JC)
        # st5[pk, jt, kh, dr, j] = r2_8[c*JC + 128*jt + j, kh*256 + dr*128 + pk]
        st5 = r2_8[jsl].reshape(NJT, 128, 2, 2, 128).transpose(4, 0, 2, 3, 1)
        if PERF_MODE == "dri":
            # SwInterleave layout: per weight load the 256 bytes are
            # [A127, B127, A126, B126, ..., A0, B0] (A = dr0, B = dr1,
            # columns reversed)
            sti = np.empty((128, NJT, 2, 256), dtype=F8)
            sti[..., 0::2] = st5[..., 0, ::-1]
            sti[..., 1::2] = st5[..., 1, ::-1]
            sti = sti.reshape(128, NJT, 2, 2, 128)
        else:
            sti = np.ascontiguousarray(st5)
        # i-rotation: physical row ip holds logical row (ip + c*JC) % B, so
        # the self diagonal of j-tile jt sits at rows jt*128 + p on all cores
        iperm = (np.arange(B) + c * JC) % B
        xr = x8[iperm]                                         # [B, D] rotated
        # xt8[pk, iq, kh, dr, i] = xr[iq*IW + i, kh*256 + dr*128 + pk]
        xt8 = np.ascontiguousarray(
            xr.reshape(NIQ, IW, 2, 2, 128).transpose(4, 0, 2, 3, 1)
        )
        rsqT = np.ascontiguousarray(
            (-rsq[jsl].reshape(NJT, 128).T).astype(np.float32)
        )
        in_maps.append({"st8": sti, "xt8": xt8, "rsqT": rsqT, "eyes": eyes})
    return in_maps


def _combine(outs, feature_ts, feature_image1, feature_image2):
    ts = np.asarray(feature_ts, dtype=np.float64)
    im1 = np.asarray(feature_image1, dtype=np.float64)
    im2 = np.asarray(feature_image2, dtype=np.float64)
    l1 = np.sqrt(((ts - im1) ** 2).sum(1))
    l2 = np.sqrt(((ts - im2) ** 2).sum(1))
    d12 = np.sqrt(((im1 - im2 + 1e-6) ** 2).sum(1))
    xsq = (ts ** 2).sum(1)

    smax = np.full(B, -np.inf)
    for c, (oE, oO) in enumerate(outs):
        s = np.maximum(
            np.asarray(oE, dtype=np.float64).max(axis=0),
            np.asarray(oO, dtype=np.float64).max(axis=0),
        )                                                       # [B] physical i
        phys = (np.arange(B) + c * JC) % B
        np.maximum.at(smax, phys, s)
    negsq = xsq - smax
    neg = np.sqrt(np.maximum(negsq, 0.0))
    trip = np.maximum(l1 + l2 + d12 - neg + 0.1, 0.0) + np.maximum(l1, l2)
    return np.float32(trip.sum() / B)


def kernel(feature_ts, feature_image1, feature_image2, _trace=False):
    global _NC_CACHE, _NC_CACHE_MODE, LAST_RESULTS
    if _NC_CACHE is None or _NC_CACHE_MODE != PERF_MODE:
        _NC_CACHE = _build_nc(PERF_MODE)
        _NC_CACHE_MODE = PERF_MODE
    if _trace:
        _install_ntff_hook()
    in_maps = _host_inputs(feature_ts, feature_image1, feature_image2)
    res = run_bass_kernel_spmd(_NC_CACHE, in_maps, list(range(M)), trace=_trace)
    LAST_RESULTS = res
    return _combine(
        [(res.results[c]["oE"], res.results[c]["oO"]) for c in range(M)],
        feature_ts, feature_image1, feature_image2,
    )
